# revision 1
# baseline (speedup 1.0000x reference)
"""Trainium2 Bass kernel for nn_Block_78993038508729 (dense transformer
block: rmsnorm -> causal MHA (+degenerate rope) -> rmsnorm -> top-2 MoE
with SwiGLU experts).

Strategy (8 NeuronCores):
  Launch A: attention, tensor-parallel over heads (2 heads/core). Each core
    computes rmsnorm(x), projects its q/k/v head slices, runs causal
    softmax attention, and emits its partial contribution of y @ wo.
    Host sums the 8 partials (TP unshard) and adds the residual.
  Host: rmsnorm2 + router + exact top-2 + per-expert token gather
    (routing is data-dependent; gather/scatter is host-side unshard work).
  Launch B: experts, expert-parallel (expert e on core e). Each core runs
    silu(tok@gate)*(tok@up) @ down for its expert's tokens, scaled by the
    routing weight. Host scatter-adds results (exact: non-selected experts
    have weight exactly 0 in the reference).

Note on rope: the reference's rope slices freqs[:NH] and broadcasts over
the sequence axis, so the rotation for each head is constant across
positions and identical for q and k. A fixed rotation applied to both
operands of a dot product cancels (orthogonal transform), so attention
scores -- and therefore the block output -- are unchanged by skipping it.

Attention matmuls use float32r (TF32-like PE mode: 1 cycle/row at N>=256,
~2^-11 relative precision -- keeps router logits accurate so the top-2
expert choice matches the reference). The MoE runs in bf16 (fast weight
loads; routing is already decided so precision only affects the expert
outputs, which are small relative to the residual).

Hardware constraint found empirically: PE transposes with different base
partitions must not be interleaved into the same PSUM tile -- the PE
wedges (NRT_EXEC_UNIT_UNRECOVERABLE). Transposes are grouped per base.
"""

import sys

if "/opt/trn_rl_repo" not in sys.path:
    sys.path.insert(0, "/opt/trn_rl_repo")

import math

import ml_dtypes
import numpy as np

import concourse.bass as bass
import concourse.mybir as mybir
import concourse.tile as tile
from concourse import bacc
from concourse.bass_utils import run_bass_kernel_spmd

F32 = mybir.dt.float32
F32R = mybir.dt.float32r
BF16 = mybir.dt.bfloat16
AF = mybir.ActivationFunctionType
BF16_NP = ml_dtypes.bfloat16

B, T, D = 1, 2048, 1024
NH, HD = 16, 64
E, K, H = 8, 2, 2048
LAYER_DEPTH = 12
EPS = 1e-8
NCORES = 8
HPC = NH // NCORES          # heads per core = 2
CW = HPC * HD               # per-core head-column width = 128
CAP = 640                   # token capacity per expert core (launch B)
MOE_SCALE = 1.0 / math.sqrt(LAYER_DEPTH)

_CACHE: dict = {}

# CoreSim doesn't implement the Silu activation; when True, build
# silu(g) as g*sigmoid(g) (identical formula, LUT-free path).
SIM_COMPAT = False


def _bacc(n_cores):
    return bacc.Bacc("TRN2", target_bir_lowering=False, debug=False,
                     num_devices=n_cores)


# --------------------------------------------------------------------------
# Launch A: attention (head-sharded).
# Per-core inputs:
#   x      [T, D]   f32   full input (replicated)
#   wqkv   [D, 3*CW] f32r  [wq_c | wk_c | wv_c] columns for this core's heads
#   bqkv   [3, CW]  f32   row 0 bq_c, row 1 bk_c, row 2 bv_c
#   wo     [CW, D]  f32r  wo rows for this core's head columns
#   trimask [128, 128] f32  triu mask: m[tk, u] = 1 iff u >= tk
#   ident  [128, 128] f32r  identity (PE transpose)
#   ident2 [128, 64] f32r  eye(64) stacked twice (per-head transposes)
#   onesc  [1, 128]  f32r  ones row (K=1 broadcast matmuls)
# Output:
#   part   [T, D]   f32   this core's partial of y @ wo (normalized)
# --------------------------------------------------------------------------

def build_attn():
    nc = _bacc(NCORES)
    x_d = nc.dram_tensor("x", [T, D], F32, kind="ExternalInput")
    wqkv_d = nc.dram_tensor("wqkv", [D, 3 * CW], F32R, kind="ExternalInput")
    bqkv_d = nc.dram_tensor("bqkv", [3, CW], F32, kind="ExternalInput")
    wo_d = nc.dram_tensor("wo", [CW, D], F32R, kind="ExternalInput")
    trimask_d = nc.dram_tensor("trimask", [128, 128], F32, kind="ExternalInput")
    ident_d = nc.dram_tensor("ident", [128, 128], F32R, kind="ExternalInput")
    ident2_d = nc.dram_tensor("ident2", [128, 64], F32R, kind="ExternalInput")
    onesc_d = nc.dram_tensor("onesc", [1, 128], F32R, kind="ExternalInput")
    part_d = nc.dram_tensor("part", [T, D], F32, kind="ExternalOutput")

    NT = T // 128            # token tiles
    NJ = T // 512            # big token blocks
    NC = D // 128            # contraction chunks

    with tile.TileContext(nc, num_cores=NCORES) as tc:
        with (
            tc.tile_pool(name="const", bufs=1) as const,
            tc.tile_pool(name="xin", bufs=2) as xin,
            tc.tile_pool(name="stat", bufs=4) as stat,
            tc.tile_pool(name="ht", bufs=1) as htp,
            tc.tile_pool(name="qkv", bufs=1) as qkvp,
            tc.tile_pool(name="expp", bufs=6) as expp,
            tc.tile_pool(name="yout", bufs=4) as youtp,
            tc.tile_pool(name="ps", bufs=2, space="PSUM") as ps,
            tc.tile_pool(name="psy", bufs=2, space="PSUM") as psy,
        ):
            ident = const.tile([128, 128], F32R)
            nc.sync.dma_start(out=ident[:], in_=ident_d[:, :])
            ident2 = const.tile([128, 64], F32R)
            nc.sync.dma_start(out=ident2[:], in_=ident2_d[:, :])
            onesc = const.tile([1, 128], F32R)
            nc.sync.dma_start(out=onesc[:], in_=onesc_d[:, :])
            trimask = const.tile([128, 128], F32)
            nc.sync.dma_start(out=trimask[:], in_=trimask_d[:, :])
            wqkv = const.tile([128, NC, 3 * CW], F32R)
            nc.sync.dma_start(
                out=wqkv[:], in_=wqkv_d.ap().rearrange("(c p) m -> p c m", p=128))
            bqkv = const.tile([128, 3], F32)
            nc.sync.dma_start(
                out=bqkv[:], in_=bqkv_d.ap().rearrange("r m -> m r"))
            wo = const.tile([128, D], F32R)
            nc.sync.dma_start(out=wo[:], in_=wo_d[:, :])
            ones16 = const.tile([128, NT], F32)
            nc.vector.memset(ones16[:], 1.0)
            epst = const.tile([128, 1], F32)
            nc.vector.memset(epst[:], EPS)

            hT = htp.tile([128, NC, T], F32R)
            qT = qkvp.tile([128, T], F32R)
            kT = qkvp.tile([128, T], F32R)
            vT = qkvp.tile([128, T], F32R)
            vext = [youtp.tile([128, NT, HD + 1], F32R, name=f"vext{h}", bufs=1)
                    for h in range(HPC)]
            yT = qkvp.tile([128, T], F32R)
            dens = [qkvp.tile([1, T], F32R, name=f"den{h}") for h in range(HPC)]

            for h in range(HPC):
                nc.scalar.activation(out=vext[h][:, :, HD], in_=ones16[:],
                                     func=AF.Copy)

            def phase123(j):
                """rmsnorm + transpose + qkv projection + v transpose for
                token block j (512 tokens = 4 tiles)."""
                jsl = bass.ts(j, 512)
                for i in range(4 * j, 4 * j + 4):
                    xt = xin.tile([128, D], F32)
                    nc.sync.dma_start(out=xt[:],
                                      in_=x_d[i * 128:(i + 1) * 128, :])
                    ssum = stat.tile([128, 1], F32)
                    sq = stat.tile([128, D], F32, bufs=2)
                    nc.scalar.activation(out=sq[:], in_=xt[:], func=AF.Square,
                                         accum_out=ssum[:])
                    rstd = stat.tile([128, 1], F32)
                    nc.scalar.activation(out=rstd[:], in_=ssum[:],
                                         func=AF.Sqrt, scale=1.0 / D,
                                         bias=epst[:])
                    nc.vector.reciprocal(out=rstd[:], in_=rstd[:])
                    h = xin.tile([128, D], F32R)
                    nc.vector.tensor_scalar_mul(h[:], xt[:], rstd[:])
                    for c2 in range(0, NC, 4):
                        ptr = ps.tile([128, 512], F32R, tag="tr")
                        for c in range(c2, c2 + 4):
                            nc.tensor.transpose(
                                ptr[:, (c - c2) * 128:(c - c2 + 1) * 128],
                                h[:, c * 128:(c + 1) * 128], ident[:])
                        dst = hT[:, c2:c2 + 4, i * 128:(i + 1) * 128]
                        src = ptr[:].rearrange("p (c q) -> p c q", q=128)
                        if (i + c2) % 2 == 0:
                            nc.scalar.copy(dst, src)
                        else:
                            nc.vector.tensor_copy(dst, src)
                # qkv projections for this block
                for out_t, col0, brow in ((qT, 0, 0), (kT, CW, 1),
                                          (vT, 2 * CW, 2)):
                    pmm = psy.tile([128, 512], F32, tag="yacc")
                    for c in range(NC):
                        nc.tensor.matmul(
                            pmm[:], wqkv[:, c, col0:col0 + CW],
                            hT[:, c, jsl], start=(c == 0), stop=(c == NC - 1))
                    if brow < 2:
                        nc.vector.tensor_scalar_add(out_t[:, jsl], pmm[:],
                                                    bqkv[:, brow:brow + 1])
                    else:
                        nc.scalar.activation(out=out_t[:, jsl], in_=pmm[:],
                                             func=AF.Identity,
                                             bias=bqkv[:, brow:brow + 1])
                # v transposes for this block (grouped per head: PE wedges on
                # mixed-base transposes within one psum tile)
                for h in range(HPC):
                    ptr = ps.tile([128, 256], F32R, tag="trv")
                    for i in range(4 * j, 4 * j + 4):
                        slot = (i - 4 * j) * 64
                        nc.tensor.transpose(
                            ptr[:, slot:slot + 64],
                            vT[h * HD:(h + 1) * HD, i * 128:(i + 1) * 128],
                            ident2[h * HD:(h + 1) * HD, :])
                    nc.vector.tensor_copy(
                        vext[h][:, 4 * j:4 * j + 4, 0:HD],
                        ptr[:].rearrange("p (i d) -> p i d", d=64))

            def attention(jq):
                """causal attention for query block jq, both heads."""
                jsl = bass.ts(jq, 512)
                for h in range(HPC):
                    hsl = slice(h * HD, (h + 1) * HD)
                    pacc = psy.tile([128, 512], F32, tag="yacc")
                    nblk = 4 * jq + 4
                    for ib in range(nblk):
                        off = (ib - 4 * jq) * 128 if ib >= 4 * jq else 0
                        pss = ps.tile([128, 512], F32, tag="scores")
                        nc.tensor.matmul(
                            pss[:, off:512],
                            kT[hsl, ib * 128:(ib + 1) * 128],
                            qT[hsl, jsl][:, off:512], start=True, stop=True)
                        et = expp.tile([128, 512], F32R, tag="exp")
                        nc.scalar.activation(out=et[:, off:512],
                                             in_=pss[:, off:512],
                                             func=AF.Exp,
                                             scale=1.0 / math.sqrt(HD))
                        if ib >= 4 * jq:
                            # triangular boundary strip
                            nc.vector.tensor_mul(et[:, off:off + 128],
                                                 et[:, off:off + 128],
                                                 trimask[:])
                        nc.tensor.matmul(
                            pacc[0:HD + 1, off:512], vext[h][:, ib, :],
                            et[:, off:512],
                            start=(ib == 0), stop=(ib == nblk - 1))
                    if h == 0:
                        nc.scalar.copy(yT[hsl, jsl], pacc[0:HD, :])
                    else:
                        nc.vector.tensor_copy(yT[hsl, jsl], pacc[0:HD, :])
                    nc.vector.tensor_copy(dens[h][0:1, jsl],
                                          pacc[HD:HD + 1, :])

            for j in range(NJ):
                phase123(j)
                attention(j)

            # ---- normalize yT by denominators ----
            with nc.allow_low_precision(
                    reason="f32r rounding of softmax denominator "
                           "reciprocals (~2^-11) is negligible"):
                for h in range(HPC):
                    nc.vector.reciprocal(out=dens[h][:], in_=dens[h][:])
            for h in range(HPC):
                hsl = slice(h * HD, (h + 1) * HD)
                for j in range(NJ):
                    jsl = bass.ts(j, 512)
                    pbd = psy.tile([128, 512], F32, tag="yacc")
                    nc.tensor.matmul(pbd[:], onesc[:], dens[h][0:1, jsl],
                                     start=True, stop=True)
                    nc.vector.tensor_mul(yT[hsl, jsl], yT[hsl, jsl],
                                         pbd[hsl, :])

            # ---- partial output projection: part = yTn.T @ wo ----
            for i in range(NT):
                for half in range(2):
                    pso = psy.tile([128, 512], F32, tag="yacc")
                    nc.tensor.matmul(
                        pso[:], yT[:, i * 128:(i + 1) * 128],
                        wo[:, half * 512:(half + 1) * 512],
                        start=True, stop=True)
                    ot = youtp.tile([128, 512], F32, tag="out")
                    if (i + half) % 2 == 0:
                        nc.scalar.copy(ot[:], pso[:])
                    else:
                        nc.vector.tensor_copy(ot[:], pso[:])
                    nc.sync.dma_start(
                        out=part_d[i * 128:(i + 1) * 128,
                                   half * 512:(half + 1) * 512],
                        in_=ot[:])
    nc.compile()
    return nc


# --------------------------------------------------------------------------
# Launch B: one expert per core (bf16 matmuls, fp32 accumulation).
# Per-core inputs:
#   tokT [D, CAP]  bf16  gathered+normed tokens (transposed), zero-padded
#   gu   [D, 2H]   bf16  [gate | up] for this core's expert
#   down [H, D]    bf16  down projection
#   wts  [CAP/128, 128] f32  routing weight * MOE_SCALE per slot (0 for pads)
# Output:
#   eout [CAP, D]  f32   weighted expert output per slot
# --------------------------------------------------------------------------

def build_moe():
    nc = _bacc(NCORES)
    tokT_d = nc.dram_tensor("tokT", [D, CAP], BF16, kind="ExternalInput")
    gu_d = nc.dram_tensor("gu", [D, 2 * H], BF16, kind="ExternalInput")
    down_d = nc.dram_tensor("down", [H, D], BF16, kind="ExternalInput")
    wts_d = nc.dram_tensor("wts", [CAP // 128, 128], F32, kind="ExternalInput")
    eout_d = nc.dram_tensor("eout", [CAP, D], F32, kind="ExternalOutput")

    NC = D // 128            # 8 d chunks
    NHT = H // 128           # 16 h tiles
    NTT = CAP // 128         # 5 token tiles

    with tile.TileContext(nc, num_cores=NCORES) as tc:
        with (
            tc.tile_pool(name="const", bufs=1) as const,
            tc.tile_pool(name="wstream", bufs=3) as wstream,
            tc.tile_pool(name="gup", bufs=1) as gup,
            tc.tile_pool(name="outp", bufs=4) as outp,
            tc.tile_pool(name="ps", bufs=2, space="PSUM") as ps,
            tc.tile_pool(name="psu", bufs=2, space="PSUM") as psu,
        ):
            tokT = const.tile([128, NC, CAP], BF16)
            nc.sync.dma_start(
                out=tokT[:], in_=tokT_d.ap().rearrange("(c p) n -> p c n", p=128))
            wts = const.tile([128, NTT], F32)
            nc.sync.dma_start(out=wts[:], in_=wts_d.ap().rearrange("t p -> p t"))
            down = const.tile([128, NHT, D], BF16)
            nc.sync.dma_start(
                out=down[:], in_=down_d.ap().rearrange("(t p) m -> p t m", p=128))

            guT = gup.tile([128, NHT, CAP], BF16)
            for t in range(NHT):
                gw = wstream.tile([128, NC, 128], BF16, tag="gw")
                nc.sync.dma_start(
                    out=gw[:],
                    in_=gu_d.ap()[:, t * 128:(t + 1) * 128]
                    .rearrange("(c p) m -> p c m", p=128))
                uw = wstream.tile([128, NC, 128], BF16, tag="uw")
                nc.sync.dma_start(
                    out=uw[:],
                    in_=gu_d.ap()[:, H + t * 128:H + (t + 1) * 128]
                    .rearrange("(c p) m -> p c m", p=128))
                for n0, n1 in ((0, 512), (512, CAP)):
                    psg = ps.tile([128, 512], F32, tag="g")
                    psuu = psu.tile([128, 512], F32, tag="u")
                    nw = n1 - n0
                    for c in range(NC):
                        nc.tensor.matmul(psg[:, 0:nw], gw[:, c, :],
                                         tokT[:, c, n0:n1],
                                         start=(c == 0), stop=(c == NC - 1))
                    for c in range(NC):
                        nc.tensor.matmul(psuu[:, 0:nw], uw[:, c, :],
                                         tokT[:, c, n0:n1],
                                         start=(c == 0), stop=(c == NC - 1))
                    sg = outp.tile([128, 512], F32, tag="sg")
                    if SIM_COMPAT:
                        nc.scalar.activation(out=sg[:, 0:nw], in_=psg[:, 0:nw],
                                             func=AF.Sigmoid)
                        nc.vector.tensor_mul(sg[:, 0:nw], sg[:, 0:nw],
                                             psg[:, 0:nw])
                    else:
                        nc.scalar.activation(out=sg[:, 0:nw], in_=psg[:, 0:nw],
                                             func=AF.Silu)
                    nc.vector.tensor_mul(guT[:, t, n0:n1], sg[:, 0:nw],
                                         psuu[:, 0:nw])

            for tt in range(NTT):
                for half in range(2):
                    pso = ps.tile([128, 512], F32, tag="o")
                    for t in range(NHT):
                        nc.tensor.matmul(
                            pso[:], guT[:, t, tt * 128:(tt + 1) * 128],
                            down[:, t, half * 512:(half + 1) * 512],
                            start=(t == 0), stop=(t == NHT - 1))
                    ot = outp.tile([128, 512], F32, tag="ot")
                    nc.vector.tensor_scalar_mul(ot[:], pso[:],
                                                wts[:, tt:tt + 1])
                    nc.sync.dma_start(
                        out=eout_d[tt * 128:(tt + 1) * 128,
                                   half * 512:(half + 1) * 512],
                        in_=ot[:])
    nc.compile()
    return nc


# --------------------------------------------------------------------------
# Host orchestration
# --------------------------------------------------------------------------

def _get(name, builder):
    if name not in _CACHE:
        _CACHE[name] = builder()
    return _CACHE[name]


def _attn_inputs(x2d, wq, bq, wkv, bkv, wo, norm1_w):
    """Build the 8 per-core input maps for launch A."""
    # fold norm1_w into the projection rows
    wq_s = wq * norm1_w[:, None]
    wkv_s = wkv * norm1_w[:, None]
    wk_s = wkv_s[:, :D]
    wv_s = wkv_s[:, D:]
    bk = bkv[:D]
    bv = bkv[D:]

    tk = np.arange(128)[:, None]
    u = np.arange(128)[None, :]
    trimask = (u >= tk).astype(np.float32)
    ident = np.eye(128, dtype=np.float32)
    ident2 = np.concatenate([np.eye(64, dtype=np.float32)] * 2, axis=0)
    onesc = np.ones((1, 128), np.float32)

    ins = []
    for c in range(NCORES):
        cs = slice(c * CW, (c + 1) * CW)
        wqkv_c = np.ascontiguousarray(
            np.concatenate([wq_s[:, cs], wk_s[:, cs], wv_s[:, cs]], axis=1))
        bqkv_c = np.ascontiguousarray(
            np.stack([bq[cs], bk[cs], bv[cs]], axis=0))
        wo_c = np.ascontiguousarray(wo[cs, :])
        ins.append({
            "x": x2d,
            "wqkv": wqkv_c,
            "bqkv": bqkv_c,
            "wo": wo_c,
            "trimask": trimask,
            "ident": ident,
            "ident2": ident2,
            "onesc": onesc,
        })
    return ins


def _route(x2, router_w, norm2_w):
    """Exact reference routing on host: rmsnorm2 + top-2 + softmax."""
    h2 = x2 / np.sqrt(np.mean(x2 * x2, axis=-1, keepdims=True) + EPS)
    h2 = (h2 * norm2_w).astype(np.float32)
    logits = h2.astype(np.float32) @ router_w.astype(np.float32)   # [N, E]
    idx1 = np.argmax(logits, axis=-1)
    l2 = logits.copy()
    l2[np.arange(T), idx1] = -np.inf
    idx2 = np.argmax(l2, axis=-1)
    v1 = logits[np.arange(T), idx1]
    v2 = logits[np.arange(T), idx2]
    # softmax over the two selected logits (v1 >= v2)
    e2 = np.exp((v2 - v1).astype(np.float32))
    p1 = (1.0 / (1.0 + e2)).astype(np.float32)
    p2 = (e2 / (1.0 + e2)).astype(np.float32)
    return h2, idx1, idx2, p1, p2


def kernel(x, freqs_cos, freqs_sin, norm1_w, wq, bq, wkv, bkv, wo, bo,
           norm2_w, router_w, gate_w, up_w, down_w):
    x = np.asarray(x, np.float32)
    x2d = np.ascontiguousarray(x.reshape(T, D))
    wq = np.asarray(wq, np.float32)
    wkv = np.asarray(wkv, np.float32)
    wo = np.asarray(wo, np.float32)
    bq = np.asarray(bq, np.float32)
    bkv = np.asarray(bkv, np.float32)
    bo = np.asarray(bo, np.float32)
    norm1_w = np.asarray(norm1_w, np.float32)
    norm2_w = np.asarray(norm2_w, np.float32)
    router_w = np.asarray(router_w, np.float32)
    gate_w = np.asarray(gate_w, np.float32)
    up_w = np.asarray(up_w, np.float32)
    down_w = np.asarray(down_w, np.float32)

    # ---- launch A ----
    nc_a = _get("attn", build_attn)
    ins_a = _attn_inputs(x2d, wq, bq, wkv, bkv, wo, norm1_w)
    res_a = run_bass_kernel_spmd(nc_a, ins_a, core_ids=list(range(NCORES)))
    parts = np.stack([res_a.results[c]["part"] for c in range(NCORES)])
    x2 = (x2d.astype(np.float64) + parts.sum(axis=0, dtype=np.float64)
          + bo.astype(np.float64)).astype(np.float32)

    # ---- host routing ----
    h2, idx1, idx2, p1, p2 = _route(x2, router_w, norm2_w)

    # per-expert token lists (order: top-1 hits then top-2 hits, stable)
    work = []   # (expert, token_idx array, weight array)
    for e in range(E):
        m1 = idx1 == e
        m2 = idx2 == e
        toks = np.concatenate([np.nonzero(m1)[0], np.nonzero(m2)[0]])
        wgts = np.concatenate([p1[m1], p2[m2]]).astype(np.float32)
        for s in range(0, len(toks), CAP):
            work.append((e, toks[s:s + CAP], wgts[s:s + CAP]))

    h2b = h2.astype(BF16_NP)
    gub: dict = {}
    downb: dict = {}

    # ---- launch B (usually one round of 8) ----
    nc_b = _get("moe", build_moe)
    moe = np.zeros((T, D), np.float64)
    for r0 in range(0, len(work), NCORES):
        batch = work[r0:r0 + NCORES]
        while len(batch) < NCORES:
            batch.append((0, np.zeros(0, np.int64), np.zeros(0, np.float32)))
        ins_b = []
        for e, toks, wgts in batch:
            tokT = np.zeros((D, CAP), BF16_NP)
            tokT[:, :len(toks)] = h2b[toks].T
            wts = np.zeros((CAP,), np.float32)
            wts[:len(toks)] = wgts * MOE_SCALE
            if e not in gub:
                gub[e] = np.ascontiguousarray(np.concatenate(
                    [gate_w[e], up_w[e]], axis=1).astype(BF16_NP))
                downb[e] = np.ascontiguousarray(down_w[e].astype(BF16_NP))
            ins_b.append({
                "tokT": tokT,
                "gu": gub[e],
                "down": downb[e],
                "wts": np.ascontiguousarray(wts.reshape(CAP // 128, 128)),
            })
        res_b = run_bass_kernel_spmd(nc_b, ins_b, core_ids=list(range(NCORES)))
        for (e, toks, wgts), rc in zip(batch, res_b.results):
            if len(toks):
                moe[toks] += rc["eout"][:len(toks)].astype(np.float64)

    out = (x2.astype(np.float64) + moe).astype(np.float32)
    return out.reshape(B, T, D)



# revision 26
# speedup vs baseline: 2.3953x; 2.3953x over previous
"""Trainium2 Bass kernel for nn_Block_78993038508729 (dense transformer
block: rmsnorm -> causal MHA (+degenerate rope) -> rmsnorm -> top-2 MoE
with SwiGLU experts).

Strategy (8 NeuronCores, two launches; host does the O(T*D) elementwise
glue between them -- norms, routing, gathers, residual adds):

  Launch A (attention, bf16): tensor-parallel over heads, 2 heads/core.
    The host precomputes hT = rmsnorm(x)*norm1_w transposed to [D, T]
    (so no device-side rmsnorm, no PE transposes, no sqrt/square
    activation-table ping-pong).  Each core projects its q/k head
    columns into [hd, tok] layout and v directly into [tok, hd] layout,
    runs causal softmax attention with the denominators carried as an
    extra ones-column through the AV matmul, and emits its partial of
    y @ wo in bf16.  Host sums the 8 partials and adds the residual.

  Host: rmsnorm2 + router + exact top-2 + per-expert token gather
    (routing is data-dependent; this is unshard/shard work).

  Launch B (experts, fp8 DoubleRow): expert-parallel, one expert/core.
    Tokens and weights are pre-quantized to fp8e4m3 on the host and
    packed in DoubleRow pair layout [128, 2, .] so every matmul runs at
    2 rows/cycle.  silu on Act, g*u on DVE (fp8 out), down-projection
    also DoubleRow.  Host scatter-adds the weighted expert outputs.

Note on rope: the reference's rope slices freqs[:NH] and broadcasts over
the sequence axis, so the rotation for each head is constant across
positions and identical for q and k.  A fixed orthogonal rotation
applied to both operands of a dot product cancels, so attention scores
-- and therefore the block output -- are unchanged by skipping it.

Numerics (validated against the reference inputs offline): bf16
attention + fp8 MoE gives rel err ~3e-3 vs the 2e-2 gate.  fp8 anywhere
in the attention path perturbs x2 enough to flip top-2 routing picks,
so attention stays bf16.
"""

import sys

if "/opt/trn_rl_repo" not in sys.path:
    sys.path.insert(0, "/opt/trn_rl_repo")

import math

import ml_dtypes
import numpy as np

import concourse.bass as bass
import concourse.mybir as mybir
import concourse.tile as tile
from concourse import bacc
from concourse.bass_utils import run_bass_kernel_spmd

F32 = mybir.dt.float32
BF16 = mybir.dt.bfloat16
F8 = mybir.dt.float8e4
AF = mybir.ActivationFunctionType
PM = mybir.MatmulPerfMode
BF16_NP = ml_dtypes.bfloat16
F8_NP = ml_dtypes.float8_e4m3fn

B, T, D = 1, 2048, 1024
NH, HD = 16, 64
E, K, H = 8, 2, 2048
LAYER_DEPTH = 12
EPS = 1e-8
NCORES = 8
HPC = NH // NCORES          # heads per core = 2
CW = HPC * HD               # per-core head-column width = 128
CAP = 576                   # token capacity per expert core (max load 547)
MOE_SCALE = 1.0 / math.sqrt(LAYER_DEPTH)

_CACHE: dict = {}
MOE_ROUNDS = 0              # launches of the moe kernel in the last call


def _bacc(n_cores):
    return bacc.Bacc("TRN2", target_bir_lowering=False, debug=False,
                     num_devices=n_cores)


# --------------------------------------------------------------------------
# Launch A: attention (head-sharded, bf16).
# Per-core inputs:
#   hT    [128, 8, T] bf16  normed input transposed: hT[p,c,t]=h[t,128c+p]
#   wqkv  [128, 8, 384] bf16  [wq_c | wk_c | wv_c] for this core's heads,
#                             wqkv[p,c,m] = W[128c+p, m]
#   bqk   [128, 2] f32      col 0 bq_c, col 1 bk_c
#   wo    [128, D] bf16     wo rows for this core's head columns
#   trimask [128, 128] bf16 m[k, q] = 1 iff q >= k
#   onesb [1, 64] bf16      ones row (denominator broadcast outer product)
# Output:
#   part  [T, D] bf16       this core's partial of y @ wo (normalized)
# --------------------------------------------------------------------------

def build_attn():
    nc = _bacc(NCORES)
    hT_d = nc.dram_tensor("hT", [128, D // 128, T], BF16, kind="ExternalInput")
    wqkv_d = nc.dram_tensor("wqkv", [128, D // 128, 3 * CW], BF16,
                            kind="ExternalInput")
    bqk_d = nc.dram_tensor("bqk", [128, 2], F32, kind="ExternalInput")
    wo_d = nc.dram_tensor("wo", [128, D], BF16, kind="ExternalInput")
    trimask_d = nc.dram_tensor("trimask", [128, 128], BF16,
                               kind="ExternalInput")
    onesb_d = nc.dram_tensor("onesb", [1, 64], BF16, kind="ExternalInput")
    part_d = nc.dram_tensor("part", [T, D], BF16, kind="ExternalOutput")

    NC = D // 128            # contraction chunks = 8
    NJ = T // 512            # query blocks = 4

    with tile.TileContext(nc, num_cores=NCORES) as tc:
        with (
            tc.tile_pool(name="const", bufs=1) as const,
            tc.tile_pool(name="big", bufs=1) as bigp,
            tc.tile_pool(name="et", bufs=3) as etp,
            tc.tile_pool(name="dens", bufs=2) as densp,
            tc.tile_pool(name="out", bufs=3) as outp,
            tc.tile_pool(name="ss", bufs=2, space="PSUM") as ps_s,
            tc.tile_pool(name="pa", bufs=2, space="PSUM") as ps_a,
            tc.tile_pool(name="mm", bufs=2, space="PSUM") as ps_m,
        ):
            # DMA issue order matters: the single DMA-engine pool serves
            # transfers in order, and the first q projection needs wqkv +
            # the first hT block before anything else.
            wqkv = const.tile([128, NC, 3 * CW], BF16)
            nc.sync.dma_start(out=wqkv[:, :, 0:CW], in_=wqkv_d[:, :, 0:CW])
            hT = bigp.tile([128, NC, T], BF16)
            nc.sync.dma_start(out=hT[:, :, 0:512], in_=hT_d[:, :, 0:512])
            nc.sync.dma_start(out=wqkv[:, :, CW:2 * CW],
                              in_=wqkv_d[:, :, CW:2 * CW])
            nc.sync.dma_start(out=wqkv[:, :, 2 * CW:3 * CW],
                              in_=wqkv_d[:, :, 2 * CW:3 * CW])
            bqk = const.tile([128, 2], F32)
            nc.sync.dma_start(out=bqk[:], in_=bqk_d[:, :])
            trimask = const.tile([128, 128], BF16)
            nc.sync.dma_start(out=trimask[:], in_=trimask_d[:, :])
            onesb = const.tile([1, 64], BF16)
            nc.sync.dma_start(out=onesb[:], in_=onesb_d[:, :])
            for j in range(1, NJ):
                jsl = bass.ts(j, 512)
                nc.sync.dma_start(out=hT[:, :, jsl], in_=hT_d[:, :, jsl])
            wo = const.tile([128, D], BF16)
            nc.sync.dma_start(out=wo[:], in_=wo_d[:, :])

            qT = bigp.tile([128, T], BF16)
            kT = bigp.tile([128, T], BF16)
            yT = bigp.tile([128, T], BF16)
            # v in [tok, hd] layout, grouped [head, 65] with a ones column
            # at local col 64 of each head group (softmax denominators).
            vdir = bigp.tile([128, T // 128, HPC, HD + 1], BF16)
            nc.vector.memset(vdir[:, :, :, HD], 1.0)

            def proj_qk(j, which):
                """q or k projection for token block j (one chunk)."""
                jsl = bass.ts(j, 512)
                dst, col0, brow = ((qT, 0, 0), (kT, CW, 1))[which]
                pq = ps_m.tile([128, 512], F32, tag="mm")
                for c in range(NC):
                    nc.tensor.matmul(pq[:], wqkv[:, c, col0:col0 + CW],
                                     hT[:, c, jsl],
                                     start=(c == 0), stop=(c == NC - 1))
                nc.vector.tensor_scalar_add(dst[:, jsl], pq[:],
                                            bqk[:, brow:brow + 1])

            def proj_v(i):
                """v projection for token tile i, directly in [tok, hd]."""
                isl = bass.ts(i, 128)
                pv = ps_m.tile([128, 512], F32, tag="mm")
                for c in range(NC):
                    nc.tensor.matmul(pv[:, 0:CW], hT[:, c, isl],
                                     wqkv[:, c, 2 * CW:3 * CW],
                                     start=(c == 0), stop=(c == NC - 1))
                nc.vector.tensor_copy(
                    vdir[:, i, :, 0:HD],
                    pv[:, 0:CW].rearrange("p (h d) -> p h d", d=HD))

            def qk_chunks(j):
                return [lambda j=j: proj_qk(j, 0), lambda j=j: proj_qk(j, 1)]

            def v_chunks(j):
                return [lambda i=i: proj_v(i) for i in range(4 * j, 4 * j + 4)]

            def outproj_chunk(i, engines=("v", "v")):
                """partial output projection + writeback for token tile i.
                Two [128,512] psum halves on the small-matmul ring so the
                scores ring is never blocked behind output copies."""
                ot = outp.tile([128, 1024], BF16, tag="ot")
                for half in range(2):
                    po = ps_m.tile([128, 512], F32, tag="mm")
                    nc.tensor.matmul(
                        po[:], yT[:, bass.ts(i, 128)],
                        wo[:, 512 * half:512 * (half + 1)],
                        start=True, stop=True)
                    dst = ot[:, 512 * half:512 * (half + 1)]
                    if engines[half] == "v":
                        nc.vector.tensor_copy(dst, po[:])
                    else:
                        nc.scalar.copy(dst, po[:])
                nc.sync.dma_start(out=part_d[bass.ts(i, 128), :], in_=ot[:])

            def outproj_chunks(j):
                return [lambda i=i: outproj_chunk(i)
                        for i in range(4 * j, 4 * j + 4)]

            def attention(j, fillers):
                """causal attention for query block j, both heads.

                Software-pipelined: the scores+exp of pair i+1 are emitted
                before the AV matmuls of pair i, so the PE always has
                score work queued while the Act engine runs exp.  The
                `fillers` (next block's projections, previous block's
                output projection) are spread between pairs to soak up
                the PE idle time while Act works through the exps.
                """
                jsl = bass.ts(j, 512)
                nblk = 4 * j + 4
                # head-interleaved: consecutive items accumulate into
                # different pacc tiles, so their chains overlap.
                items = [(h, ib0) for ib0 in range(0, nblk, 2)
                         for h in range(HPC)]
                paccs = {}
                ets = {}

                def stage_scores(h, ib0):
                    hsl = slice(h * HD, (h + 1) * HD)
                    if ib0 == 0:
                        paccs[h] = ps_a.tile([HD + 1, 512], F32, tag="pacc",
                                             name=f"pacc{h}")
                    pss = ps_s.tile([128, 1024], F32, tag="ss")
                    et = etp.tile([128, 1024], BF16, tag="et")
                    ets[(h, ib0)] = et
                    offs = []
                    for half, ib in enumerate((ib0, ib0 + 1)):
                        off = max(0, (ib - 4 * j) * 128)
                        offs.append(off)
                        nc.tensor.matmul(
                            pss[:, 512 * half + off:512 * (half + 1)],
                            kT[hsl, bass.ts(ib, 128)],
                            qT[hsl, jsl][:, off:512],
                            start=True, stop=True)
                    nc.scalar.activation(
                        out=et[:, offs[0]:1024], in_=pss[:, offs[0]:1024],
                        func=AF.Exp, scale=1.0 / math.sqrt(HD))
                    for half, ib in enumerate((ib0, ib0 + 1)):
                        off = offs[half]
                        if ib >= 4 * j:  # triangular boundary strip (Pool)
                            nc.gpsimd.tensor_mul(
                                et[:, 512 * half + off:512 * half + off + 128],
                                et[:, 512 * half + off:512 * half + off + 128],
                                trimask[:])

                def stage_av(h, ib0):
                    hsl = slice(h * HD, (h + 1) * HD)
                    pacc = paccs[h]
                    et = ets.pop((h, ib0))
                    for half, ib in enumerate((ib0, ib0 + 1)):
                        off = max(0, (ib - 4 * j) * 128)
                        nc.tensor.matmul(
                            pacc[:, off:512], vdir[:, ib, h, :],
                            et[:, 512 * half + off:512 * (half + 1)],
                            start=(ib == 0), stop=(ib == nblk - 1))
                    if ib0 + 2 >= nblk:
                        # normalize: yT = pacc[0:64] * (1/den); the
                        # reciprocal row is broadcast across partitions by
                        # the (otherwise idle) GPSIMD engine, and the bf16
                        # multiply then runs in the DVE 2x mode.
                        dr = densp.tile([1, 512], BF16, tag="dr")
                        with nc.allow_low_precision(
                                reason="bf16 rounding of softmax denominator"
                                       " reciprocals (~0.4%) is negligible"):
                            nc.vector.reciprocal(out=dr[:],
                                                 in_=pacc[HD:HD + 1, :])
                        # full-height broadcast so the multiply's operands
                        # share a start partition (HW TensorTensor rule)
                        drb = densp.tile([128, 512], BF16, tag="drb")
                        nc.gpsimd.partition_broadcast(drb[:], dr[0:1, :])
                        nc.scalar.copy(yT[hsl, jsl], pacc[0:HD, :])
                        nc.vector.tensor_mul(yT[hsl, jsl], yT[hsl, jsl],
                                             drb[hsl, :])

                stage_scores(*items[0])
                n = len(items)
                total = len(fillers)
                done = 0
                for i in range(n):
                    if i + 1 < n:
                        stage_scores(*items[i + 1])
                    stage_av(*items[i])
                    target = -(-total * (i + 1) // n)  # ceil fair share
                    while done < target:
                        fillers[done]()
                        done += 1

            # Block 0's q/k/v run up front.  After that, each block's v
            # projections ride as early fillers of its own attention (the
            # diagonal AV tiles that need them come last), while the next
            # block's q/k and the previous block's output projection fill
            # the rest of the Act-bound stretches.
            for f in qk_chunks(0) + v_chunks(0):
                f()
            for j in range(NJ):
                fill = []
                if j >= 1:
                    fill += v_chunks(j)
                if j + 1 < NJ:
                    fill += qk_chunks(j + 1)
                if j >= 1:
                    fill += outproj_chunks(j - 1)
                attention(j, fill)
            # final block's output projection: the scores ring is free by
            # now, so use its wide tiles, with the two half-copies split
            # across both copy engines to shorten the tail.
            for i in range(4 * (NJ - 1), 4 * NJ):
                po = ps_s.tile([128, 1024], F32, tag="ss")
                for half in range(2):
                    nc.tensor.matmul(
                        po[:, 512 * half:512 * (half + 1)],
                        yT[:, bass.ts(i, 128)],
                        wo[:, 512 * half:512 * (half + 1)],
                        start=True, stop=True)
                ot = outp.tile([128, 1024], BF16, tag="ot")
                nc.vector.tensor_copy(ot[:, 0:512], po[:, 0:512])
                nc.scalar.copy(ot[:, 512:1024], po[:, 512:1024])
                nc.sync.dma_start(out=part_d[bass.ts(i, 128), :], in_=ot[:])
    nc.compile()
    return nc


# --------------------------------------------------------------------------
# Launch B: one expert per core (fp8e4m3 DoubleRow matmuls, f32 psum).
# Per-core inputs:
#   tok8 [128, 8, CAP] fp8   gathered+normed tokens: tok8[p,c,n]=h2[n,128c+p]
#   guw  [16, 128, 8, 256] fp8  per h-tile t: [:,:,0:128]=gate cols,
#                               [:,:,128:256]=up cols, d-major pairs
#   dwn8 [128, 8, 2, D] fp8  down: dwn8[p,hp,i,m]=down[256hp+128i+p, m]
#   wts  [128, 5] f32        routing weight * MOE_SCALE per slot (0 pads)
# Output:
#   eout [CAP, D] bf16       weighted expert output per slot
# --------------------------------------------------------------------------

def build_moe():
    nc = _bacc(NCORES)
    NHT = H // 128           # 16 h tiles
    NTT = (CAP + 127) // 128  # 5 token tiles (last one 64 wide)
    tok8_d = nc.dram_tensor("tok8", [128, D // 128, CAP], F8,
                            kind="ExternalInput")
    guw_d = nc.dram_tensor("guw", [NHT, 128, D // 128, 256], F8,
                           kind="ExternalInput")
    dwn8_d = nc.dram_tensor("dwn8", [128, H // 256, 2, D], F8,
                            kind="ExternalInput")
    wts_d = nc.dram_tensor("wts", [128, NTT], F32, kind="ExternalInput")
    eout_d = nc.dram_tensor("eout", [CAP, D], BF16, kind="ExternalOutput")

    NC2 = D // 256           # 4 DoubleRow d-chunks

    with tile.TileContext(nc, num_cores=NCORES) as tc:
        with (
            tc.tile_pool(name="const", bufs=1) as const,
            tc.tile_pool(name="wstream", bufs=6) as wstream,
            tc.tile_pool(name="gup", bufs=1) as gup,
            tc.tile_pool(name="sg", bufs=2) as sgp,
            tc.tile_pool(name="outp", bufs=3) as outp,
            tc.tile_pool(name="pgu", bufs=2, space="PSUM") as pgu,
            tc.tile_pool(name="po", bufs=2, space="PSUM") as po_p,
        ):
            dwn8 = const.tile([128, H // 256, 2, D], F8)
            guT = gup.tile([128, NHT, CAP], F8)
            tok8 = const.tile([128, D // 128, CAP], F8)
            wts = const.tile([128, NTT], F32)

            for t in range(NHT):
                gw = wstream.tile([128, D // 128, 256], F8, tag="gw")
                nc.sync.dma_start(out=gw[:], in_=guw_d[t, :, :, :])
                if t == 0:
                    # first gate/up tile, then tokens in two halves (the
                    # first d-chunks land before the first matmuls want
                    # them), then the rest of the stream; the 2MB down
                    # weights go last -- they're not needed until the
                    # second phase and would stall the gate/up stream.
                    nc.sync.dma_start(out=tok8[:, 0:4, :],
                                      in_=tok8_d[:, 0:4, :])
                    nc.sync.dma_start(out=tok8[:, 4:8, :],
                                      in_=tok8_d[:, 4:8, :])
                    nc.sync.dma_start(out=wts[:], in_=wts_d[:, :])
                if t == NHT - 1:
                    nc.sync.dma_start(out=dwn8[:], in_=dwn8_d[:, :, :, :])
                # g/u psum: [0:512]=g, [512:1024]=u for the first 512
                # tokens (bank-aligned); tail 64 tokens in a second tile.
                pwA = pgu.tile([128, 1024], F32, tag="guA")
                pwB = pgu.tile([128, 128], F32, tag="guB")
                for gu in range(2):
                    csl = slice(gu * 128, gu * 128 + 128)
                    for c in range(NC2):
                        nc.tensor.matmul(
                            pwA[:, gu * 512:gu * 512 + 512],
                            gw[:, 2 * c:2 * c + 2, csl],
                            tok8[:, 2 * c:2 * c + 2, 0:512],
                            start=(c == 0), stop=(c == NC2 - 1),
                            perf_mode=PM.DoubleRow)
                    for c in range(NC2):
                        nc.tensor.matmul(
                            pwB[:, gu * 64:gu * 64 + 64],
                            gw[:, 2 * c:2 * c + 2, csl],
                            tok8[:, 2 * c:2 * c + 2, 512:CAP],
                            start=(c == 0), stop=(c == NC2 - 1),
                            perf_mode=PM.DoubleRow)
                sg = sgp.tile([128, CAP], BF16, tag="sg")
                nc.scalar.activation(out=sg[:, 0:512], in_=pwA[:, 0:512],
                                     func=AF.Silu)
                nc.scalar.activation(out=sg[:, 512:CAP], in_=pwB[:, 0:64],
                                     func=AF.Silu)
                nc.vector.tensor_mul(guT[:, t, 0:512], sg[:, 0:512],
                                     pwA[:, 512:1024])
                nc.vector.tensor_mul(guT[:, t, 512:CAP], sg[:, 512:CAP],
                                     pwB[:, 64:128])

            for tt in range(NTT):
                ntok = min(128, CAP - tt * 128)
                tsl = slice(tt * 128, tt * 128 + ntok)
                ot = outp.tile([128, D], BF16, tag="ot")
                for half in range(2):
                    dsl = slice(half * 512, half * 512 + 512)
                    pso = po_p.tile([128, 512], F32, tag="o")
                    for hp in range(H // 256):
                        nc.tensor.matmul(
                            pso[0:ntok, :], guT[:, 2 * hp:2 * hp + 2, tsl],
                            dwn8[:, hp, :, dsl],
                            start=(hp == 0), stop=(hp == H // 256 - 1),
                            perf_mode=PM.DoubleRow)
                    nc.vector.tensor_scalar_mul(ot[0:ntok, dsl],
                                                pso[0:ntok, :],
                                                wts[0:ntok, tt:tt + 1])
                    nc.sync.dma_start(out=eout_d[tsl, dsl],
                                      in_=ot[0:ntok, dsl])
    nc.compile()
    return nc


# --------------------------------------------------------------------------
# Host orchestration
# --------------------------------------------------------------------------

def _get(name, builder):
    if name not in _CACHE:
        _CACHE[name] = builder()
    return _CACHE[name]


def _attn_inputs(x2d, wq, bq, wkv, bkv, wo, norm1_w):
    """Build the 8 per-core input maps for launch A."""
    h = x2d.astype(np.float64)
    h = h / np.sqrt((h * h).mean(axis=-1, keepdims=True) + EPS)
    h = (h * norm1_w.astype(np.float64)).astype(np.float32)
    # hT[p, c, t] = h[t, 128c+p]
    hT = np.ascontiguousarray(
        h.T.reshape(D // 128, 128, T).transpose(1, 0, 2).astype(BF16_NP))

    wk = wkv[:, :D]
    wv = wkv[:, D:]
    bk = bkv[:D]

    tk = np.arange(128)[:, None]
    u = np.arange(128)[None, :]
    trimask = (u >= tk).astype(BF16_NP)
    onesb = np.ones((1, 64), BF16_NP)

    ins = []
    for c in range(NCORES):
        cs = slice(c * CW, (c + 1) * CW)
        wqkv_c = np.concatenate([wq[:, cs], wk[:, cs], wv[:, cs]], axis=1)
        wqkv_c = np.ascontiguousarray(
            wqkv_c.reshape(D // 128, 128, 3 * CW).transpose(1, 0, 2)
            .astype(BF16_NP))
        bqk_c = np.ascontiguousarray(
            np.stack([bq[cs], bk[cs]], axis=1).astype(np.float32))
        wo_c = np.ascontiguousarray(wo[cs, :].astype(BF16_NP))
        ins.append({
            "hT": hT,
            "wqkv": wqkv_c,
            "bqk": bqk_c,
            "wo": wo_c,
            "trimask": trimask,
            "onesb": onesb,
        })
    return ins


def _route(x2, router_w, norm2_w):
    """Exact reference routing on host: rmsnorm2 + top-2 + softmax."""
    h2 = x2 / np.sqrt(np.mean(x2 * x2, axis=-1, keepdims=True) + EPS)
    h2 = (h2 * norm2_w).astype(np.float32)
    logits = h2.astype(np.float32) @ router_w.astype(np.float32)   # [N, E]
    idx1 = np.argmax(logits, axis=-1)
    l2 = logits.copy()
    l2[np.arange(T), idx1] = -np.inf
    idx2 = np.argmax(l2, axis=-1)
    v1 = logits[np.arange(T), idx1]
    v2 = logits[np.arange(T), idx2]
    # softmax over the two selected logits (v1 >= v2)
    e2 = np.exp((v2 - v1).astype(np.float32))
    p1 = (1.0 / (1.0 + e2)).astype(np.float32)
    p2 = (e2 / (1.0 + e2)).astype(np.float32)
    return h2, idx1, idx2, p1, p2


def kernel(x, freqs_cos, freqs_sin, norm1_w, wq, bq, wkv, bkv, wo, bo,
           norm2_w, router_w, gate_w, up_w, down_w):
    global MOE_ROUNDS
    x = np.asarray(x, np.float32)
    x2d = np.ascontiguousarray(x.reshape(T, D))
    wq = np.asarray(wq, np.float32)
    wkv = np.asarray(wkv, np.float32)
    wo = np.asarray(wo, np.float32)
    bq = np.asarray(bq, np.float32)
    bkv = np.asarray(bkv, np.float32)
    bo = np.asarray(bo, np.float32)
    norm1_w = np.asarray(norm1_w, np.float32)
    norm2_w = np.asarray(norm2_w, np.float32)
    router_w = np.asarray(router_w, np.float32)
    gate_w = np.asarray(gate_w, np.float32)
    up_w = np.asarray(up_w, np.float32)
    down_w = np.asarray(down_w, np.float32)

    # ---- launch A ----
    nc_a = _get("attn", build_attn)
    ins_a = _attn_inputs(x2d, wq, bq, wkv, bkv, wo, norm1_w)
    res_a = run_bass_kernel_spmd(nc_a, ins_a, core_ids=list(range(NCORES)))
    parts = np.stack([res_a.results[c]["part"].astype(np.float64)
                      for c in range(NCORES)])
    # v-bias folds through attention as +bv (softmax weights sum to 1),
    # so its wo image is added host-side along with bo.
    bv = bkv[D:].astype(np.float64)
    x2 = (x2d.astype(np.float64) + parts.sum(axis=0)
          + bv @ wo.astype(np.float64) + bo.astype(np.float64)
          ).astype(np.float32)

    # ---- host routing ----
    h2, idx1, idx2, p1, p2 = _route(x2, router_w, norm2_w)

    # per-expert token lists (order: top-1 hits then top-2 hits, stable)
    work = []   # (expert, token_idx array, weight array)
    for e in range(E):
        m1 = idx1 == e
        m2 = idx2 == e
        toks = np.concatenate([np.nonzero(m1)[0], np.nonzero(m2)[0]])
        wgts = np.concatenate([p1[m1], p2[m2]]).astype(np.float32)
        for s in range(0, max(len(toks), 1), CAP):
            work.append((e, toks[s:s + CAP], wgts[s:s + CAP]))

    h28 = h2.astype(F8_NP)
    guwb: dict = {}
    dwnb: dict = {}
    NTT = (CAP + 127) // 128

    # ---- launch B (one round of 8 unless an expert overflows CAP) ----
    nc_b = _get("moe", build_moe)
    moe = np.zeros((T, D), np.float64)
    MOE_ROUNDS = 0
    for r0 in range(0, len(work), NCORES):
        batch = work[r0:r0 + NCORES]
        while len(batch) < NCORES:
            batch.append((0, np.zeros(0, np.int64), np.zeros(0, np.float32)))
        ins_b = []
        for e, toks, wgts in batch:
            tok8 = np.zeros((128, D // 128, CAP), F8_NP)
            tok8t = h28[toks].T.reshape(D // 128, 128, len(toks))
            tok8[:, :, :len(toks)] = tok8t.transpose(1, 0, 2)
            wts = np.zeros((NTT * 128,), np.float32)
            wts[:len(toks)] = wgts * MOE_SCALE
            if e not in guwb:
                gu = np.concatenate([
                    gate_w[e].reshape(D, H // 128, 128),
                    up_w[e].reshape(D, H // 128, 128)], axis=2)  # [D,16,256]
                guwb[e] = np.ascontiguousarray(
                    gu.reshape(D // 128, 128, H // 128, 256)
                    .transpose(2, 1, 0, 3).astype(F8_NP))
                dwnb[e] = np.ascontiguousarray(
                    down_w[e].reshape(H // 256, 2, 128, D)
                    .transpose(2, 0, 1, 3).astype(F8_NP))
            ins_b.append({
                "tok8": tok8,
                "guw": guwb[e],
                "dwn8": dwnb[e],
                "wts": np.ascontiguousarray(
                    wts.reshape(NTT, 128).T.astype(np.float32)),
            })
        res_b = run_bass_kernel_spmd(nc_b, ins_b, core_ids=list(range(NCORES)))
        MOE_ROUNDS += 1
        for (e, toks, wgts), rc in zip(batch, res_b.results):
            if len(toks):
                moe[toks] += rc["eout"][:len(toks)].astype(np.float64)

    out = (x2.astype(np.float64) + moe).astype(np.float32)
    return out.reshape(B, T, D)


# revision 42
# speedup vs baseline: 2.6358x; 1.1004x over previous
"""Trainium2 Bass kernel for nn_Block_78993038508729 (dense transformer
block: rmsnorm -> causal MHA (+degenerate rope) -> rmsnorm -> top-2 MoE
with SwiGLU experts).

Strategy (8 NeuronCores, two launches; host does the O(T*D) elementwise
glue between them -- norms, routing, gathers, residual adds):

  Launch A (attention, bf16): tensor-parallel over heads, 2 heads/core.
    The host precomputes hT = rmsnorm(x)*norm1_w transposed to [D, T]
    (so no device-side rmsnorm, no PE transposes, no sqrt/square
    activation-table ping-pong).  Each core projects its q/k head
    columns into [hd, tok] layout and v directly into [tok, hd] layout,
    runs causal softmax attention with the denominators carried as an
    extra ones-column through the AV matmul, and emits its partial of
    y @ wo in bf16.  Host sums the 8 partials and adds the residual.

  Host: rmsnorm2 + router + exact top-2 + per-expert token gather
    (routing is data-dependent; this is unshard/shard work).

  Launch B (experts, fp8 DoubleRow): expert-parallel, one expert/core.
    Tokens and weights are pre-quantized to fp8e4m3 on the host and
    packed in DoubleRow pair layout [128, 2, .] so every matmul runs at
    2 rows/cycle.  silu on Act, g*u on DVE (fp8 out), down-projection
    also DoubleRow.  Host scatter-adds the weighted expert outputs.

Note on rope: the reference's rope slices freqs[:NH] and broadcasts over
the sequence axis, so the rotation for each head is constant across
positions and identical for q and k.  A fixed orthogonal rotation
applied to both operands of a dot product cancels, so attention scores
-- and therefore the block output -- are unchanged by skipping it.

Numerics (validated against the reference inputs offline): bf16
attention + fp8 MoE gives rel err ~3e-3 vs the 2e-2 gate.  fp8 anywhere
in the attention path perturbs x2 enough to flip top-2 routing picks,
so attention stays bf16.
"""

import sys

if "/opt/trn_rl_repo" not in sys.path:
    sys.path.insert(0, "/opt/trn_rl_repo")

import math

import ml_dtypes
import numpy as np

import concourse.bass as bass
import concourse.mybir as mybir
import concourse.tile as tile
from concourse import bacc
from concourse.bass_utils import run_bass_kernel_spmd

F32 = mybir.dt.float32
BF16 = mybir.dt.bfloat16
F8 = mybir.dt.float8e4
AF = mybir.ActivationFunctionType
PM = mybir.MatmulPerfMode
BF16_NP = ml_dtypes.bfloat16
F8_NP = ml_dtypes.float8_e4m3fn

B, T, D = 1, 2048, 1024
NH, HD = 16, 64
E, K, H = 8, 2, 2048
LAYER_DEPTH = 12
EPS = 1e-8
NCORES = 8
HPC = NH // NCORES          # heads per core = 2
CW = HPC * HD               # per-core head-column width = 128
CAP = 576                   # token capacity per expert core (max load 547)
MOE_SCALE = 1.0 / math.sqrt(LAYER_DEPTH)

_CACHE: dict = {}
MOE_ROUNDS = 0              # launches of the moe kernel in the last call


def _bacc(n_cores):
    return bacc.Bacc("TRN2", target_bir_lowering=False, debug=False,
                     num_devices=n_cores)


# --------------------------------------------------------------------------
# Launch A: attention (head-sharded, bf16).
# Per-core inputs:
#   hT    [128, 8, T] bf16  normed input transposed: hT[p,c,t]=h[t,128c+p]
#   wqkv  [128, 8, 384] bf16  [wq_c | wk_c | wv_c] for this core's heads,
#                             wqkv[p,c,m] = W[128c+p, m]
#   bqk   [128, 2] f32      col 0 bq_c, col 1 bk_c
#   wo    [128, D] bf16     wo rows for this core's head columns
#   trimask [128, 128] bf16 m[k, q] = 1 iff q >= k
#   onesb [1, 64] bf16      ones row (denominator broadcast outer product)
# Output:
#   part  [T, D] bf16       this core's partial of y @ wo (normalized)
# --------------------------------------------------------------------------

def build_attn():
    nc = _bacc(NCORES)
    hT_d = nc.dram_tensor("hT", [128, D // 128, T], BF16, kind="ExternalInput")
    w_d = {w: nc.dram_tensor(w, [128, D // 128, CW], BF16,
                             kind="ExternalInput") for w in ("wq", "wk", "wv")}
    bqk_d = nc.dram_tensor("bqk", [128, 2], F32, kind="ExternalInput")
    wo_d = nc.dram_tensor("wo", [128, D], BF16, kind="ExternalInput")
    trimask_d = nc.dram_tensor("trimask", [128, 128], BF16,
                               kind="ExternalInput")
    onesb_d = nc.dram_tensor("onesb", [1, 128], BF16, kind="ExternalInput")
    part_d = nc.dram_tensor("part", [T, D], BF16, kind="ExternalOutput")

    NC = D // 128            # contraction chunks = 8
    NJ = T // 512            # query blocks = 4

    with tile.TileContext(nc, num_cores=NCORES) as tc:
        with (
            tc.tile_pool(name="const", bufs=1) as const,
            tc.tile_pool(name="big", bufs=1) as bigp,
            tc.tile_pool(name="et", bufs=4) as etp,
            tc.tile_pool(name="dens", bufs=2) as densp,
            tc.tile_pool(name="out", bufs=6) as outp,
            tc.tile_pool(name="ss", bufs=2, space="PSUM") as ps_s,
            tc.tile_pool(name="pa", bufs=2, space="PSUM") as ps_a,
            tc.tile_pool(name="mm", bufs=2, space="PSUM") as ps_m,
        ):
            # DMA issue order matters: the single DMA-engine pool serves
            # transfers in order, and the first q projection needs the q
            # weights + the first hT block before anything else.
            wqkv = {w: const.tile([128, NC, CW], BF16, name=w)
                    for w in ("wq", "wk", "wv")}
            nc.sync.dma_start(out=wqkv["wq"][:], in_=w_d["wq"][:, :, :])
            hT = bigp.tile([128, NC, T], BF16)
            nc.sync.dma_start(out=hT[:, 0:4, 0:512], in_=hT_d[:, 0:4, 0:512])
            nc.sync.dma_start(out=hT[:, 4:8, 0:512], in_=hT_d[:, 4:8, 0:512])
            nc.sync.dma_start(out=wqkv["wk"][:], in_=w_d["wk"][:, :, :])
            nc.sync.dma_start(out=wqkv["wv"][:], in_=w_d["wv"][:, :, :])
            bqk = const.tile([128, 2], F32)
            nc.sync.dma_start(out=bqk[:], in_=bqk_d[:, :])
            trimask = const.tile([128, 128], BF16)
            nc.sync.dma_start(out=trimask[:], in_=trimask_d[:, :])
            onesb = const.tile([1, 128], BF16)
            nc.sync.dma_start(out=onesb[:], in_=onesb_d[:, :])
            for j in range(1, NJ):
                jsl = bass.ts(j, 512)
                nc.sync.dma_start(out=hT[:, :, jsl], in_=hT_d[:, :, jsl])
            wo = const.tile([128, D], BF16)
            nc.sync.dma_start(out=wo[:], in_=wo_d[:, :])

            qT = bigp.tile([128, T], BF16)
            kT = bigp.tile([128, T], BF16)
            yT = bigp.tile([128, T], BF16)
            # v in [tok, hd] layout, grouped [head, 65] with a ones column
            # at local col 64 of each head group (softmax denominators).
            vdir = bigp.tile([128, T // 128, HPC, HD + 1], BF16)
            nc.vector.memset(vdir[:, :, :, HD], 1.0)

            def proj_qk(j, which):
                """q or k projection for token block j (one chunk)."""
                jsl = bass.ts(j, 512)
                dst, wname, brow = ((qT, "wq", 0), (kT, "wk", 1))[which]
                pq = ps_m.tile([128, 512], F32, tag="mm")
                for c in range(NC):
                    nc.tensor.matmul(pq[:], wqkv[wname][:, c, :],
                                     hT[:, c, jsl],
                                     start=(c == 0), stop=(c == NC - 1))
                nc.vector.tensor_scalar_add(dst[:, jsl], pq[:],
                                            bqk[:, brow:brow + 1])

            def proj_v(i):
                """v projection for token tile i, directly in [tok, hd]."""
                isl = bass.ts(i, 128)
                pv = ps_m.tile([128, 512], F32, tag="mm")
                for c in range(NC):
                    nc.tensor.matmul(pv[:, 0:CW], hT[:, c, isl],
                                     wqkv["wv"][:, c, :],
                                     start=(c == 0), stop=(c == NC - 1))
                nc.vector.tensor_copy(
                    vdir[:, i, :, 0:HD],
                    pv[:, 0:CW].rearrange("p (h d) -> p h d", d=HD))

            def qk_chunks(j):
                return [lambda j=j: proj_qk(j, 0), lambda j=j: proj_qk(j, 1)]

            def v_chunks(j):
                return [lambda i=i: proj_v(i) for i in range(4 * j, 4 * j + 4)]

            def outproj_chunk(i, engines=("v", "v")):
                """partial output projection + writeback for token tile i.
                Two [128,512] psum halves on the small-matmul ring so the
                scores ring is never blocked behind output copies."""
                ot = outp.tile([128, 1024], BF16, tag="ot")
                for half in range(2):
                    po = ps_m.tile([128, 512], F32, tag="mm")
                    nc.tensor.matmul(
                        po[:], yT[:, bass.ts(i, 128)],
                        wo[:, 512 * half:512 * (half + 1)],
                        start=True, stop=True)
                    dst = ot[:, 512 * half:512 * (half + 1)]
                    if engines[half] == "v":
                        nc.vector.tensor_copy(dst, po[:])
                    else:
                        nc.scalar.copy(dst, po[:])
                nc.sync.dma_start(out=part_d[bass.ts(i, 128), :], in_=ot[:])

            def outproj_chunks(j):
                return [lambda i=i: outproj_chunk(i)
                        for i in range(4 * j, 4 * j + 4)]

            def attention(j, fillers):
                """causal attention for query block j, both heads.

                Software-pipelined: the scores+exp of pair i+1 are emitted
                before the AV matmuls of pair i, so the PE always has
                score work queued while the Act engine runs exp.  The
                `fillers` (next block's projections, previous block's
                output projection) are spread between pairs to soak up
                the PE idle time while Act works through the exps.
                """
                jsl = bass.ts(j, 512)
                nblk = 4 * j + 4
                # head-interleaved: consecutive items accumulate into
                # different pacc tiles, so their chains overlap.
                items = [(h, ib0) for ib0 in range(0, nblk, 2)
                         for h in range(HPC)]
                paccs = {}
                ets = {}

                def stage_scores(h, ib0):
                    hsl = slice(h * HD, (h + 1) * HD)
                    if ib0 == 0:
                        paccs[h] = ps_a.tile([HD + 1, 512], F32, tag="pacc",
                                             name=f"pacc{h}")
                    pss = ps_s.tile([128, 1024], F32, tag="ss")
                    et = etp.tile([128, 1024], BF16, tag="et")
                    ets[(h, ib0)] = et
                    offs = []
                    for half, ib in enumerate((ib0, ib0 + 1)):
                        off = max(0, (ib - 4 * j) * 128)
                        offs.append(off)
                        nc.tensor.matmul(
                            pss[:, 512 * half + off:512 * (half + 1)],
                            kT[hsl, bass.ts(ib, 128)],
                            qT[hsl, jsl][:, off:512],
                            start=True, stop=True)
                    nc.scalar.activation(
                        out=et[:, offs[0]:1024], in_=pss[:, offs[0]:1024],
                        func=AF.Exp, scale=1.0 / math.sqrt(HD))
                    for half, ib in enumerate((ib0, ib0 + 1)):
                        off = offs[half]
                        if ib >= 4 * j:  # triangular boundary strip (Pool)
                            nc.gpsimd.tensor_mul(
                                et[:, 512 * half + off:512 * half + off + 128],
                                et[:, 512 * half + off:512 * half + off + 128],
                                trimask[:])

                def stage_av(h, ib0):
                    hsl = slice(h * HD, (h + 1) * HD)
                    pacc = paccs[h]
                    et = ets.pop((h, ib0))
                    for half, ib in enumerate((ib0, ib0 + 1)):
                        off = max(0, (ib - 4 * j) * 128)
                        nc.tensor.matmul(
                            pacc[:, off:512], vdir[:, ib, h, :],
                            et[:, 512 * half + off:512 * (half + 1)],
                            start=(ib == 0), stop=(ib == nblk - 1))
                    if ib0 + 2 >= nblk:
                        # normalize: yT = pacc[0:64] * (1/den); the
                        # reciprocal row is broadcast across partitions by
                        # the (otherwise idle) GPSIMD engine -- except in
                        # the final block, where the PE is idle and its
                        # outer-product broadcast has lower latency.
                        dr = densp.tile([1, 512], BF16, tag="dr")
                        with nc.allow_low_precision(
                                reason="bf16 rounding of softmax denominator"
                                       " reciprocals (~0.4%) is negligible"):
                            nc.vector.reciprocal(out=dr[:],
                                                 in_=pacc[HD:HD + 1, :])
                        if j == NJ - 1:
                            nc.scalar.copy(yT[hsl, jsl], pacc[0:HD, :])
                        else:
                            nc.vector.tensor_copy(yT[hsl, jsl],
                                                  pacc[0:HD, :])
                        if j == NJ - 1:
                            pbd = ps_m.tile([128, 512], F32, tag="mm")
                            nc.tensor.matmul(pbd[:], onesb[:], dr[:],
                                             start=True, stop=True)
                            nc.vector.tensor_mul(yT[hsl, jsl],
                                                 yT[hsl, jsl], pbd[hsl, :])
                        else:
                            drb = densp.tile([128, 512], BF16, tag="drb")
                            nc.gpsimd.partition_broadcast(drb[:], dr[0:1, :])
                            nc.vector.tensor_mul(yT[hsl, jsl],
                                                 yT[hsl, jsl], drb[hsl, :])

                n = len(items)
                for w in range(min(2, n)):
                    stage_scores(*items[w])
                total = len(fillers)
                done = 0
                for i in range(n):
                    if i + 2 < n:
                        stage_scores(*items[i + 2])
                    target = -(-total * (i + 1) // n)  # ceil fair share
                    while done < target:
                        fillers[done]()
                        done += 1
                    stage_av(*items[i])

            # Block 0's q/k/v run up front.  After that, each block's v
            # projections ride as early fillers of its own attention (the
            # diagonal AV tiles that need them come last), while the next
            # block's q/k and the previous block's output projection fill
            # the rest of the Act-bound stretches.
            for f in qk_chunks(0) + v_chunks(0):
                f()
            for j in range(NJ):
                fill = []
                if j >= 1:
                    fill += v_chunks(j)
                if j + 1 < NJ:
                    fill += qk_chunks(j + 1)
                if j >= 1:
                    fill += outproj_chunks(j - 1)
                attention(j, fill)
            # final block's output projection: both psum rings are free
            # by now, so rotate tiles across them (4-deep pipeline), with
            # the half-copies alternating between both copy engines and
            # per-half DMA writebacks to shorten the tail.
            for i in range(4 * (NJ - 1), 4 * NJ):
                ot = outp.tile([128, 1024], BF16, tag="ot")
                if i % 2 == 0:
                    pow_ = ps_s.tile([128, 1024], F32, tag="ss")
                    pos = [pow_[:, 0:512], pow_[:, 512:1024]]
                else:
                    pos = [ps_m.tile([128, 512], F32, tag="mm",
                                     name=f"poa{i}")[:],
                           ps_m.tile([128, 512], F32, tag="mm",
                                     name=f"pob{i}")[:]]
                for half in range(2):
                    nc.tensor.matmul(
                        pos[half], yT[:, bass.ts(i, 128)],
                        wo[:, 512 * half:512 * (half + 1)],
                        start=True, stop=True)
                    dst = ot[:, 512 * half:512 * (half + 1)]
                    if (i + half) % 2 == 0:
                        nc.vector.tensor_copy(dst, pos[half])
                    else:
                        nc.scalar.copy(dst, pos[half])
                    nc.sync.dma_start(
                        out=part_d[bass.ts(i, 128),
                                   512 * half:512 * (half + 1)],
                        in_=dst)
    nc.compile()
    return nc


# --------------------------------------------------------------------------
# Launch B: one expert per core (fp8e4m3 DoubleRow matmuls, f32 psum).
# Per-core inputs:
#   tok8 [128, 8, CAP] fp8   gathered+normed tokens: tok8[p,c,n]=h2[n,128c+p]
#   guw  [16, 128, 8, 256] fp8  per h-tile t: [:,:,0:128]=gate cols,
#                               [:,:,128:256]=up cols, d-major pairs
#   dwn8 [128, 8, 2, D] fp8  down: dwn8[p,hp,i,m]=down[256hp+128i+p, m]
#   wts  [128, 5] f32        routing weight * MOE_SCALE per slot (0 pads)
# Output:
#   eout [CAP, D] bf16       weighted expert output per slot
# --------------------------------------------------------------------------

def build_moe():
    nc = _bacc(NCORES)
    NHT = H // 128           # 16 h tiles
    NTT = (CAP + 127) // 128  # 5 token tiles (last one 64 wide)
    tok8_d = nc.dram_tensor("tok8", [128, D // 128, CAP], F8,
                            kind="ExternalInput")
    guw_d = nc.dram_tensor("guw", [NHT, 128, D // 128, 256], F8,
                           kind="ExternalInput")
    dwn8_d = nc.dram_tensor("dwn8", [128, H // 256, 2, D], F8,
                            kind="ExternalInput")
    wts_d = nc.dram_tensor("wts", [128, NTT], F32, kind="ExternalInput")
    eout_d = nc.dram_tensor("eout", [CAP, D], BF16, kind="ExternalOutput")

    NC2 = D // 256           # 4 DoubleRow d-chunks

    with tile.TileContext(nc, num_cores=NCORES) as tc:
        with (
            tc.tile_pool(name="const", bufs=1) as const,
            tc.tile_pool(name="wstream", bufs=8) as wstream,
            tc.tile_pool(name="gup", bufs=1) as gup,
            tc.tile_pool(name="sg", bufs=2) as sgp,
            tc.tile_pool(name="outp", bufs=3) as outp,
            tc.tile_pool(name="pgu", bufs=3, space="PSUM") as pgu,
            tc.tile_pool(name="po", bufs=2, space="PSUM") as po_p,
        ):
            dwn8 = const.tile([128, H // 256, 2, D], F8)
            guT = gup.tile([128, NHT, CAP], F8)
            tok8 = const.tile([128, D // 128, CAP], F8)
            wts = const.tile([128, NTT], F32)

            # Per-tile gate/up weight DMAs (fine granularity keeps the
            # consumer from waiting on big lumps); tokens right after the
            # first tile, the 2MB down weights last -- they're not needed
            # until the second phase and would stall the gate/up stream.
            gws = []
            for t in range(NHT):
                gw = wstream.tile([128, D // 128, 256], F8, tag="gw",
                                  name=f"gw{t}")
                nc.sync.dma_start(out=gw[:], in_=guw_d[t, :, :, :])
                gws.append(gw)
                if t == 0:
                    nc.sync.dma_start(out=tok8[:, 0:4, :],
                                      in_=tok8_d[:, 0:4, :])
                    nc.sync.dma_start(out=tok8[:, 4:8, :],
                                      in_=tok8_d[:, 4:8, :])
                    nc.sync.dma_start(out=wts[:], in_=wts_d[:, :])
            nc.sync.dma_start(out=dwn8[:], in_=dwn8_d[:, :, :, :])

            for t in range(NHT):
                gw = gws[t]
                # g/u psum: [0:512]=g, [512:1024]=u for the first 512
                # tokens (3-deep ring); the 64-token tail shares the
                # down-projection ring so the main ring stays deep.
                pwA = pgu.tile([128, 1024], F32, tag="guA")
                pwB = po_p.tile([128, 512], F32, tag="o",
                                name=f"pwB{t}")[:, 0:128]
                for gu in range(2):
                    csl = slice(gu * 128, gu * 128 + 128)
                    for c in range(NC2):
                        nc.tensor.matmul(
                            pwA[:, gu * 512:gu * 512 + 512],
                            gw[:, 2 * c:2 * c + 2, csl],
                            tok8[:, 2 * c:2 * c + 2, 0:512],
                            start=(c == 0), stop=(c == NC2 - 1),
                            perf_mode=PM.DoubleRow)
                    for c in range(NC2):
                        nc.tensor.matmul(
                            pwB[:, gu * 64:gu * 64 + 64],
                            gw[:, 2 * c:2 * c + 2, csl],
                            tok8[:, 2 * c:2 * c + 2, 512:CAP],
                            start=(c == 0), stop=(c == NC2 - 1),
                            perf_mode=PM.DoubleRow)
                sg = sgp.tile([128, CAP], BF16, tag="sg")
                nc.scalar.activation(out=sg[:, 0:512], in_=pwA[:, 0:512],
                                     func=AF.Silu)
                nc.scalar.activation(out=sg[:, 512:CAP], in_=pwB[:, 0:64],
                                     func=AF.Silu)
                nc.vector.tensor_mul(guT[:, t, 0:512], sg[:, 0:512],
                                     pwA[:, 512:1024])
                nc.vector.tensor_mul(guT[:, t, 512:CAP], sg[:, 512:CAP],
                                     pwB[:, 64:128])

            for tt in range(NTT):
                ntok = min(128, CAP - tt * 128)
                tsl = slice(tt * 128, tt * 128 + ntok)
                ot = outp.tile([128, D], BF16, tag="ot")
                for half in range(2):
                    dsl = slice(half * 512, half * 512 + 512)
                    pso = po_p.tile([128, 512], F32, tag="o",
                                    name=f"pso{tt}_{half}")
                    for hp in range(H // 256):
                        nc.tensor.matmul(
                            pso[0:ntok, :], guT[:, 2 * hp:2 * hp + 2, tsl],
                            dwn8[:, hp, :, dsl],
                            start=(hp == 0), stop=(hp == H // 256 - 1),
                            perf_mode=PM.DoubleRow)
                    nc.vector.tensor_scalar_mul(ot[0:ntok, dsl],
                                                pso[0:ntok, :],
                                                wts[0:ntok, tt:tt + 1])
                    nc.sync.dma_start(out=eout_d[tsl, dsl],
                                      in_=ot[0:ntok, dsl])
    nc.compile()
    return nc


# --------------------------------------------------------------------------
# Host orchestration
# --------------------------------------------------------------------------

def _get(name, builder):
    if name not in _CACHE:
        _CACHE[name] = builder()
    return _CACHE[name]


def _attn_inputs(x2d, wq, bq, wkv, bkv, wo, norm1_w):
    """Build the 8 per-core input maps for launch A."""
    h = x2d.astype(np.float64)
    h = h / np.sqrt((h * h).mean(axis=-1, keepdims=True) + EPS)
    h = (h * norm1_w.astype(np.float64)).astype(np.float32)
    # hT[p, c, t] = h[t, 128c+p]
    hT = np.ascontiguousarray(
        h.T.reshape(D // 128, 128, T).transpose(1, 0, 2).astype(BF16_NP))

    wk = wkv[:, :D]
    wv = wkv[:, D:]
    bk = bkv[:D]

    tk = np.arange(128)[:, None]
    u = np.arange(128)[None, :]
    trimask = (u >= tk).astype(BF16_NP)
    onesb = np.ones((1, 128), BF16_NP)

    ins = []
    for c in range(NCORES):
        cs = slice(c * CW, (c + 1) * CW)
        packed = {n: np.ascontiguousarray(
            w[:, cs].reshape(D // 128, 128, CW).transpose(1, 0, 2)
            .astype(BF16_NP)) for n, w in (("wq", wq), ("wk", wk),
                                           ("wv", wv))}
        bqk_c = np.ascontiguousarray(
            np.stack([bq[cs], bk[cs]], axis=1).astype(np.float32))
        wo_c = np.ascontiguousarray(wo[cs, :].astype(BF16_NP))
        ins.append({
            "hT": hT,
            **packed,
            "bqk": bqk_c,
            "wo": wo_c,
            "trimask": trimask,
            "onesb": onesb,
        })
    return ins


def _route(x2, router_w, norm2_w):
    """Exact reference routing on host: rmsnorm2 + top-2 + softmax."""
    h2 = x2 / np.sqrt(np.mean(x2 * x2, axis=-1, keepdims=True) + EPS)
    h2 = (h2 * norm2_w).astype(np.float32)
    logits = h2.astype(np.float32) @ router_w.astype(np.float32)   # [N, E]
    idx1 = np.argmax(logits, axis=-1)
    l2 = logits.copy()
    l2[np.arange(T), idx1] = -np.inf
    idx2 = np.argmax(l2, axis=-1)
    v1 = logits[np.arange(T), idx1]
    v2 = logits[np.arange(T), idx2]
    # softmax over the two selected logits (v1 >= v2)
    e2 = np.exp((v2 - v1).astype(np.float32))
    p1 = (1.0 / (1.0 + e2)).astype(np.float32)
    p2 = (e2 / (1.0 + e2)).astype(np.float32)
    return h2, idx1, idx2, p1, p2


def kernel(x, freqs_cos, freqs_sin, norm1_w, wq, bq, wkv, bkv, wo, bo,
           norm2_w, router_w, gate_w, up_w, down_w):
    global MOE_ROUNDS
    x = np.asarray(x, np.float32)
    x2d = np.ascontiguousarray(x.reshape(T, D))
    wq = np.asarray(wq, np.float32)
    wkv = np.asarray(wkv, np.float32)
    wo = np.asarray(wo, np.float32)
    bq = np.asarray(bq, np.float32)
    bkv = np.asarray(bkv, np.float32)
    bo = np.asarray(bo, np.float32)
    norm1_w = np.asarray(norm1_w, np.float32)
    norm2_w = np.asarray(norm2_w, np.float32)
    router_w = np.asarray(router_w, np.float32)
    gate_w = np.asarray(gate_w, np.float32)
    up_w = np.asarray(up_w, np.float32)
    down_w = np.asarray(down_w, np.float32)

    # ---- launch A ----
    nc_a = _get("attn", build_attn)
    ins_a = _attn_inputs(x2d, wq, bq, wkv, bkv, wo, norm1_w)
    res_a = run_bass_kernel_spmd(nc_a, ins_a, core_ids=list(range(NCORES)))
    parts = np.stack([res_a.results[c]["part"].astype(np.float64)
                      for c in range(NCORES)])
    # v-bias folds through attention as +bv (softmax weights sum to 1),
    # so its wo image is added host-side along with bo.
    bv = bkv[D:].astype(np.float64)
    x2 = (x2d.astype(np.float64) + parts.sum(axis=0)
          + bv @ wo.astype(np.float64) + bo.astype(np.float64)
          ).astype(np.float32)

    # ---- host routing ----
    h2, idx1, idx2, p1, p2 = _route(x2, router_w, norm2_w)

    # per-expert token lists (order: top-1 hits then top-2 hits, stable)
    work = []   # (expert, token_idx array, weight array)
    for e in range(E):
        m1 = idx1 == e
        m2 = idx2 == e
        toks = np.concatenate([np.nonzero(m1)[0], np.nonzero(m2)[0]])
        wgts = np.concatenate([p1[m1], p2[m2]]).astype(np.float32)
        for s in range(0, max(len(toks), 1), CAP):
            work.append((e, toks[s:s + CAP], wgts[s:s + CAP]))

    h28 = h2.astype(F8_NP)
    guwb: dict = {}
    dwnb: dict = {}
    NTT = (CAP + 127) // 128

    # ---- launch B (one round of 8 unless an expert overflows CAP) ----
    nc_b = _get("moe", build_moe)
    moe = np.zeros((T, D), np.float64)
    MOE_ROUNDS = 0
    for r0 in range(0, len(work), NCORES):
        batch = work[r0:r0 + NCORES]
        while len(batch) < NCORES:
            batch.append((0, np.zeros(0, np.int64), np.zeros(0, np.float32)))
        ins_b = []
        for e, toks, wgts in batch:
            tok8 = np.zeros((128, D // 128, CAP), F8_NP)
            tok8t = h28[toks].T.reshape(D // 128, 128, len(toks))
            tok8[:, :, :len(toks)] = tok8t.transpose(1, 0, 2)
            wts = np.zeros((NTT * 128,), np.float32)
            wts[:len(toks)] = wgts * MOE_SCALE
            if e not in guwb:
                gu = np.concatenate([
                    gate_w[e].reshape(D, H // 128, 128),
                    up_w[e].reshape(D, H // 128, 128)], axis=2)  # [D,16,256]
                guwb[e] = np.ascontiguousarray(
                    gu.reshape(D // 128, 128, H // 128, 256)
                    .transpose(2, 1, 0, 3).astype(F8_NP))
                dwnb[e] = np.ascontiguousarray(
                    down_w[e].reshape(H // 256, 2, 128, D)
                    .transpose(2, 0, 1, 3).astype(F8_NP))
            ins_b.append({
                "tok8": tok8,
                "guw": guwb[e],
                "dwn8": dwnb[e],
                "wts": np.ascontiguousarray(
                    wts.reshape(NTT, 128).T.astype(np.float32)),
            })
        res_b = run_bass_kernel_spmd(nc_b, ins_b, core_ids=list(range(NCORES)))
        MOE_ROUNDS += 1
        for (e, toks, wgts), rc in zip(batch, res_b.results):
            if len(toks):
                moe[toks] += rc["eout"][:len(toks)].astype(np.float64)

    out = (x2.astype(np.float64) + moe).astype(np.float32)
    return out.reshape(B, T, D)


# revision 44
# speedup vs baseline: 2.6676x; 1.0121x over previous
"""Trainium2 Bass kernel for nn_Block_78993038508729 (dense transformer
block: rmsnorm -> causal MHA (+degenerate rope) -> rmsnorm -> top-2 MoE
with SwiGLU experts).

Strategy (8 NeuronCores, two launches; host does the O(T*D) elementwise
glue between them -- norms, routing, gathers, residual adds):

  Launch A (attention, bf16): tensor-parallel over heads, 2 heads/core.
    The host precomputes hT = rmsnorm(x)*norm1_w transposed to [D, T]
    (so no device-side rmsnorm, no PE transposes, no sqrt/square
    activation-table ping-pong).  Each core projects its q/k head
    columns into [hd, tok] layout and v directly into [tok, hd] layout,
    runs causal softmax attention with the denominators carried as an
    extra ones-column through the AV matmul, and emits its partial of
    y @ wo in bf16.  Host sums the 8 partials and adds the residual.

  Host: rmsnorm2 + router + exact top-2 + per-expert token gather
    (routing is data-dependent; this is unshard/shard work).

  Launch B (experts, fp8 DoubleRow): expert-parallel, one expert/core.
    Tokens and weights are pre-quantized to fp8e4m3 on the host and
    packed in DoubleRow pair layout [128, 2, .] so every matmul runs at
    2 rows/cycle.  silu on Act, g*u on DVE (fp8 out), down-projection
    also DoubleRow.  Host scatter-adds the weighted expert outputs.

Note on rope: the reference's rope slices freqs[:NH] and broadcasts over
the sequence axis, so the rotation for each head is constant across
positions and identical for q and k.  A fixed orthogonal rotation
applied to both operands of a dot product cancels, so attention scores
-- and therefore the block output -- are unchanged by skipping it.

Numerics (validated against the reference inputs offline): bf16
attention + fp8 MoE gives rel err ~3e-3 vs the 2e-2 gate.  fp8 anywhere
in the attention path perturbs x2 enough to flip top-2 routing picks,
so attention stays bf16.
"""

import sys

if "/opt/trn_rl_repo" not in sys.path:
    sys.path.insert(0, "/opt/trn_rl_repo")

import math

import ml_dtypes
import numpy as np

import concourse.bass as bass
import concourse.mybir as mybir
import concourse.tile as tile
from concourse import bacc
from concourse.bass_utils import run_bass_kernel_spmd

F32 = mybir.dt.float32
BF16 = mybir.dt.bfloat16
F8 = mybir.dt.float8e4
AF = mybir.ActivationFunctionType
PM = mybir.MatmulPerfMode
BF16_NP = ml_dtypes.bfloat16
F8_NP = ml_dtypes.float8_e4m3fn

B, T, D = 1, 2048, 1024
NH, HD = 16, 64
E, K, H = 8, 2, 2048
LAYER_DEPTH = 12
EPS = 1e-8
NCORES = 8
HPC = NH // NCORES          # heads per core = 2
CW = HPC * HD               # per-core head-column width = 128
CAP = 576                   # token capacity per expert core (max load 547)
MOE_SCALE = 1.0 / math.sqrt(LAYER_DEPTH)

_CACHE: dict = {}
MOE_ROUNDS = 0              # launches of the moe kernel in the last call


def _bacc(n_cores):
    return bacc.Bacc("TRN2", target_bir_lowering=False, debug=False,
                     num_devices=n_cores)


# --------------------------------------------------------------------------
# Launch A: attention (head-sharded, bf16).
# Per-core inputs:
#   hT    [128, 8, T] bf16  normed input transposed: hT[p,c,t]=h[t,128c+p]
#   wqkv  [128, 8, 384] bf16  [wq_c | wk_c | wv_c] for this core's heads,
#                             wqkv[p,c,m] = W[128c+p, m]
#   bqk   [128, 2] f32      col 0 bq_c, col 1 bk_c
#   wo    [128, D] bf16     wo rows for this core's head columns
#   trimask [128, 128] bf16 m[k, q] = 1 iff q >= k
#   onesb [1, 64] bf16      ones row (denominator broadcast outer product)
# Output:
#   part  [T, D] bf16       this core's partial of y @ wo (normalized)
# --------------------------------------------------------------------------

def build_attn():
    nc = _bacc(NCORES)
    hT_d = nc.dram_tensor("hT", [128, D // 128, T], BF16, kind="ExternalInput")
    w_d = {w: nc.dram_tensor(w, [128, D // 128, CW], BF16,
                             kind="ExternalInput") for w in ("wq", "wk", "wv")}
    bqk_d = nc.dram_tensor("bqk", [128, 2], F32, kind="ExternalInput")
    wo_d = nc.dram_tensor("wo", [128, D], BF16, kind="ExternalInput")
    trimask_d = nc.dram_tensor("trimask", [128, 128], BF16,
                               kind="ExternalInput")
    onesb_d = nc.dram_tensor("onesb", [1, 128], BF16, kind="ExternalInput")
    part_d = nc.dram_tensor("part", [T, D], BF16, kind="ExternalOutput")

    NC = D // 128            # contraction chunks = 8
    NJ = T // 512            # query blocks = 4

    with tile.TileContext(nc, num_cores=NCORES) as tc:
        with (
            tc.tile_pool(name="const", bufs=1) as const,
            tc.tile_pool(name="big", bufs=1) as bigp,
            tc.tile_pool(name="et", bufs=4) as etp,
            tc.tile_pool(name="dens", bufs=2) as densp,
            tc.tile_pool(name="out", bufs=6) as outp,
            tc.tile_pool(name="ss", bufs=2, space="PSUM") as ps_s,
            tc.tile_pool(name="pa", bufs=2, space="PSUM") as ps_a,
            tc.tile_pool(name="mm", bufs=2, space="PSUM") as ps_m,
        ):
            # DMA issue order matters: the single DMA-engine pool serves
            # transfers in order, and the first q projection needs the q
            # weights + the first hT block before anything else.
            wqkv = {w: const.tile([128, NC, CW], BF16, name=w)
                    for w in ("wq", "wk", "wv")}
            nc.sync.dma_start(out=wqkv["wq"][:], in_=w_d["wq"][:, :, :])
            hT = bigp.tile([128, NC, T], BF16)
            nc.sync.dma_start(out=hT[:, 0:4, 0:512], in_=hT_d[:, 0:4, 0:512])
            nc.sync.dma_start(out=hT[:, 4:8, 0:512], in_=hT_d[:, 4:8, 0:512])
            nc.sync.dma_start(out=wqkv["wk"][:], in_=w_d["wk"][:, :, :])
            nc.sync.dma_start(out=wqkv["wv"][:], in_=w_d["wv"][:, :, :])
            bqk = const.tile([128, 2], F32)
            nc.sync.dma_start(out=bqk[:], in_=bqk_d[:, :])
            trimask = const.tile([128, 128], BF16)
            nc.sync.dma_start(out=trimask[:], in_=trimask_d[:, :])
            onesb = const.tile([1, 128], BF16)
            nc.sync.dma_start(out=onesb[:], in_=onesb_d[:, :])
            for j in range(1, NJ):
                jsl = bass.ts(j, 512)
                nc.sync.dma_start(out=hT[:, :, jsl], in_=hT_d[:, :, jsl])
            wo = const.tile([128, D], BF16)
            nc.sync.dma_start(out=wo[:], in_=wo_d[:, :])

            qT = bigp.tile([128, T], BF16)
            kT = bigp.tile([128, T], BF16)
            yT = bigp.tile([128, T], BF16)
            # v in [tok, hd] layout, grouped [head, 65] with a ones column
            # at local col 64 of each head group (softmax denominators).
            vdir = bigp.tile([128, T // 128, HPC, HD + 1], BF16)
            nc.vector.memset(vdir[:, :, :, HD], 1.0)

            def proj_qk(j, which):
                """q or k projection for token block j (one chunk)."""
                jsl = bass.ts(j, 512)
                dst, wname, brow = ((qT, "wq", 0), (kT, "wk", 1))[which]
                pq = ps_m.tile([128, 512], F32, tag="mm")
                for c in range(NC):
                    nc.tensor.matmul(pq[:], wqkv[wname][:, c, :],
                                     hT[:, c, jsl],
                                     start=(c == 0), stop=(c == NC - 1))
                nc.vector.tensor_scalar_add(dst[:, jsl], pq[:],
                                            bqk[:, brow:brow + 1])

            def proj_v(i):
                """v projection for token tile i, directly in [tok, hd]."""
                isl = bass.ts(i, 128)
                pv = ps_m.tile([128, 512], F32, tag="mm")
                for c in range(NC):
                    nc.tensor.matmul(pv[:, 0:CW], hT[:, c, isl],
                                     wqkv["wv"][:, c, :],
                                     start=(c == 0), stop=(c == NC - 1))
                nc.vector.tensor_copy(
                    vdir[:, i, :, 0:HD],
                    pv[:, 0:CW].rearrange("p (h d) -> p h d", d=HD))

            def qk_chunks(j):
                return [lambda j=j: proj_qk(j, 0), lambda j=j: proj_qk(j, 1)]

            def v_chunks(j):
                return [lambda i=i: proj_v(i) for i in range(4 * j, 4 * j + 4)]

            def outproj_chunk(i, engines=("v", "v")):
                """partial output projection + writeback for token tile i.
                Two [128,512] psum halves on the small-matmul ring so the
                scores ring is never blocked behind output copies."""
                ot = outp.tile([128, 1024], BF16, tag="ot")
                for half in range(2):
                    po = ps_m.tile([128, 512], F32, tag="mm")
                    nc.tensor.matmul(
                        po[:], yT[:, bass.ts(i, 128)],
                        wo[:, 512 * half:512 * (half + 1)],
                        start=True, stop=True)
                    dst = ot[:, 512 * half:512 * (half + 1)]
                    if engines[half] == "v":
                        nc.vector.tensor_copy(dst, po[:])
                    else:
                        nc.scalar.copy(dst, po[:])
                nc.sync.dma_start(out=part_d[bass.ts(i, 128), :], in_=ot[:])

            def outproj_chunks(j):
                return [lambda i=i: outproj_chunk(i)
                        for i in range(4 * j, 4 * j + 4)]

            def attention(j, fillers):
                """causal attention for query block j, both heads.

                Software-pipelined: the scores+exp of pair i+1 are emitted
                before the AV matmuls of pair i, so the PE always has
                score work queued while the Act engine runs exp.  The
                `fillers` (next block's projections, previous block's
                output projection) are spread between pairs to soak up
                the PE idle time while Act works through the exps.
                """
                jsl = bass.ts(j, 512)
                nblk = 4 * j + 4
                # head-interleaved: consecutive items accumulate into
                # different pacc tiles, so their chains overlap.
                items = [(h, ib0) for ib0 in range(0, nblk, 2)
                         for h in range(HPC)]
                paccs = {}
                ets = {}

                def stage_scores(h, ib0):
                    hsl = slice(h * HD, (h + 1) * HD)
                    if ib0 == 0:
                        paccs[h] = ps_a.tile([HD + 1, 512], F32, tag="pacc",
                                             name=f"pacc{h}")
                    pss = ps_s.tile([128, 1024], F32, tag="ss")
                    et = etp.tile([128, 1024], BF16, tag="et")
                    ets[(h, ib0)] = et
                    offs = []
                    for half, ib in enumerate((ib0, ib0 + 1)):
                        off = max(0, (ib - 4 * j) * 128)
                        offs.append(off)
                        nc.tensor.matmul(
                            pss[:, 512 * half + off:512 * (half + 1)],
                            kT[hsl, bass.ts(ib, 128)],
                            qT[hsl, jsl][:, off:512],
                            start=True, stop=True)
                    nc.scalar.activation(
                        out=et[:, offs[0]:1024], in_=pss[:, offs[0]:1024],
                        func=AF.Exp, scale=1.0 / math.sqrt(HD))
                    for half, ib in enumerate((ib0, ib0 + 1)):
                        off = offs[half]
                        if ib >= 4 * j:  # triangular boundary strip (Pool)
                            nc.gpsimd.tensor_mul(
                                et[:, 512 * half + off:512 * half + off + 128],
                                et[:, 512 * half + off:512 * half + off + 128],
                                trimask[:])

                def stage_av(h, ib0):
                    hsl = slice(h * HD, (h + 1) * HD)
                    pacc = paccs[h]
                    et = ets.pop((h, ib0))
                    for half, ib in enumerate((ib0, ib0 + 1)):
                        off = max(0, (ib - 4 * j) * 128)
                        nc.tensor.matmul(
                            pacc[:, off:512], vdir[:, ib, h, :],
                            et[:, 512 * half + off:512 * (half + 1)],
                            start=(ib == 0), stop=(ib == nblk - 1))
                    if ib0 + 2 >= nblk:
                        if j < NJ - 1:
                            # normalize: yT = pacc[0:64] * (1/den); the
                            # reciprocal row is broadcast across partitions
                            # by the (otherwise idle) GPSIMD engine.
                            dr = densp.tile([1, 512], BF16, tag="dr")
                            with nc.allow_low_precision(
                                    reason="bf16 rounding of softmax "
                                           "denominator reciprocals is "
                                           "negligible"):
                                nc.vector.reciprocal(out=dr[:],
                                                     in_=pacc[HD:HD + 1, :])
                            nc.vector.tensor_copy(yT[hsl, jsl],
                                                  pacc[0:HD, :])
                            drb = densp.tile([128, 512], BF16, tag="drb")
                            nc.gpsimd.partition_broadcast(drb[:], dr[0:1, :])
                            nc.vector.tensor_mul(yT[hsl, jsl],
                                                 yT[hsl, jsl], drb[hsl, :])
                        else:
                            norm_pending.append((h, pacc))

                def finish_norms():
                    # final block: both heads' normalizes batched so the
                    # DVE never waits a PE round-trip (recip,recip then
                    # mul,mul), with the PE outer-product broadcast (the
                    # PE is idle here and has lower latency than GPSIMD).
                    drs = []
                    for h, pacc in norm_pending:
                        dr = densp.tile([1, 512], BF16, tag="dr",
                                        name=f"drf{h}")
                        with nc.allow_low_precision(
                                reason="bf16 rounding of softmax "
                                       "denominator reciprocals is "
                                       "negligible"):
                            nc.vector.reciprocal(out=dr[:],
                                                 in_=pacc[HD:HD + 1, :])
                        drs.append(dr)
                    for (h, pacc), dr in zip(norm_pending, drs):
                        hsl = slice(h * HD, (h + 1) * HD)
                        nc.scalar.copy(yT[hsl, jsl], pacc[0:HD, :])
                        pbd = ps_m.tile([128, 512], F32, tag="mm",
                                        name=f"pbdf{h}")
                        nc.tensor.matmul(pbd[:], onesb[:], dr[:],
                                         start=True, stop=True)
                        nc.vector.tensor_mul(yT[hsl, jsl],
                                             yT[hsl, jsl], pbd[hsl, :])

                norm_pending = []
                n = len(items)
                for w in range(min(2, n)):
                    stage_scores(*items[w])
                total = len(fillers)
                done = 0
                for i in range(n):
                    if i + 2 < n:
                        stage_scores(*items[i + 2])
                    target = -(-total * (i + 1) // n)  # ceil fair share
                    while done < target:
                        fillers[done]()
                        done += 1
                    stage_av(*items[i])
                if norm_pending:
                    finish_norms()

            # Block 0's q/k/v run up front.  After that, each block's v
            # projections ride as early fillers of its own attention (the
            # diagonal AV tiles that need them come last), while the next
            # block's q/k and the previous block's output projection fill
            # the rest of the Act-bound stretches.
            for f in qk_chunks(0) + v_chunks(0):
                f()
            for j in range(NJ):
                fill = []
                if j >= 1:
                    fill += v_chunks(j)
                if j + 1 < NJ:
                    fill += qk_chunks(j + 1)
                if j >= 1:
                    fill += outproj_chunks(j - 1)
                attention(j, fill)
            # final block's output projection: both psum rings are free
            # by now, so rotate tiles across them (4-deep pipeline), with
            # the half-copies alternating between both copy engines and
            # per-half DMA writebacks to shorten the tail.
            for i in range(4 * (NJ - 1), 4 * NJ):
                ot = outp.tile([128, 1024], BF16, tag="ot")
                if i % 2 == 0:
                    pow_ = ps_s.tile([128, 1024], F32, tag="ss")
                    pos = [pow_[:, 0:512], pow_[:, 512:1024]]
                else:
                    pos = [ps_m.tile([128, 512], F32, tag="mm",
                                     name=f"poa{i}")[:],
                           ps_m.tile([128, 512], F32, tag="mm",
                                     name=f"pob{i}")[:]]
                for half in range(2):
                    nc.tensor.matmul(
                        pos[half], yT[:, bass.ts(i, 128)],
                        wo[:, 512 * half:512 * (half + 1)],
                        start=True, stop=True)
                    dst = ot[:, 512 * half:512 * (half + 1)]
                    if (i + half) % 2 == 0:
                        nc.vector.tensor_copy(dst, pos[half])
                    else:
                        nc.scalar.copy(dst, pos[half])
                nc.sync.dma_start(out=part_d[bass.ts(i, 128), :], in_=ot[:])
    nc.compile()
    return nc


# --------------------------------------------------------------------------
# Launch B: one expert per core (fp8e4m3 DoubleRow matmuls, f32 psum).
# Per-core inputs:
#   tok8 [128, 8, CAP] fp8   gathered+normed tokens: tok8[p,c,n]=h2[n,128c+p]
#   guw  [16, 128, 8, 256] fp8  per h-tile t: [:,:,0:128]=gate cols,
#                               [:,:,128:256]=up cols, d-major pairs
#   dwn8 [128, 8, 2, D] fp8  down: dwn8[p,hp,i,m]=down[256hp+128i+p, m]
#   wts  [128, 5] f32        routing weight * MOE_SCALE per slot (0 pads)
# Output:
#   eout [CAP, D] bf16       weighted expert output per slot
# --------------------------------------------------------------------------

def build_moe():
    nc = _bacc(NCORES)
    NHT = H // 128           # 16 h tiles
    NTT = (CAP + 127) // 128  # 5 token tiles (last one 64 wide)
    tok8_d = nc.dram_tensor("tok8", [128, D // 128, CAP], F8,
                            kind="ExternalInput")
    guw_d = nc.dram_tensor("guw", [NHT, 128, D // 128, 256], F8,
                           kind="ExternalInput")
    dwn8_d = nc.dram_tensor("dwn8", [128, H // 256, 2, D], F8,
                            kind="ExternalInput")
    wts_d = nc.dram_tensor("wts", [128, NTT], F32, kind="ExternalInput")
    eout_d = nc.dram_tensor("eout", [CAP, D], BF16, kind="ExternalOutput")

    NC2 = D // 256           # 4 DoubleRow d-chunks

    with tile.TileContext(nc, num_cores=NCORES) as tc:
        with (
            tc.tile_pool(name="const", bufs=1) as const,
            tc.tile_pool(name="wstream", bufs=8) as wstream,
            tc.tile_pool(name="gup", bufs=1) as gup,
            tc.tile_pool(name="sg", bufs=2) as sgp,
            tc.tile_pool(name="outp", bufs=3) as outp,
            tc.tile_pool(name="pgu", bufs=3, space="PSUM") as pgu,
            tc.tile_pool(name="po", bufs=2, space="PSUM") as po_p,
        ):
            dwn8 = const.tile([128, H // 256, 2, D], F8)
            guT = gup.tile([128, NHT, CAP], F8)
            tok8 = const.tile([128, D // 128, CAP], F8)
            wts = const.tile([128, NTT], F32)

            # Per-tile gate/up weight DMAs (fine granularity keeps the
            # consumer from waiting on big lumps); tokens right after the
            # first tile, the 2MB down weights last -- they're not needed
            # until the second phase and would stall the gate/up stream.
            gws = []
            for t in range(NHT):
                gw = wstream.tile([128, D // 128, 256], F8, tag="gw",
                                  name=f"gw{t}")
                nc.sync.dma_start(out=gw[:], in_=guw_d[t, :, :, :])
                gws.append(gw)
                if t == 0:
                    nc.sync.dma_start(out=tok8[:, 0:4, :],
                                      in_=tok8_d[:, 0:4, :])
                    nc.sync.dma_start(out=tok8[:, 4:8, :],
                                      in_=tok8_d[:, 4:8, :])
                    nc.sync.dma_start(out=wts[:], in_=wts_d[:, :])
            nc.sync.dma_start(out=dwn8[:], in_=dwn8_d[:, :, :, :])

            for t in range(NHT):
                gw = gws[t]
                # g/u psum: [0:512]=g, [512:1024]=u for the first 512
                # tokens (3-deep ring); the 64-token tail shares the
                # down-projection ring so the main ring stays deep.
                pwA = pgu.tile([128, 1024], F32, tag="guA")
                pwB = po_p.tile([128, 512], F32, tag="o",
                                name=f"pwB{t}")[:, 0:128]
                for gu in range(2):
                    csl = slice(gu * 128, gu * 128 + 128)
                    for c in range(NC2):
                        nc.tensor.matmul(
                            pwA[:, gu * 512:gu * 512 + 512],
                            gw[:, 2 * c:2 * c + 2, csl],
                            tok8[:, 2 * c:2 * c + 2, 0:512],
                            start=(c == 0), stop=(c == NC2 - 1),
                            perf_mode=PM.DoubleRow)
                    for c in range(NC2):
                        nc.tensor.matmul(
                            pwB[:, gu * 64:gu * 64 + 64],
                            gw[:, 2 * c:2 * c + 2, csl],
                            tok8[:, 2 * c:2 * c + 2, 512:CAP],
                            start=(c == 0), stop=(c == NC2 - 1),
                            perf_mode=PM.DoubleRow)
                sg = sgp.tile([128, CAP], BF16, tag="sg")
                nc.scalar.activation(out=sg[:, 0:512], in_=pwA[:, 0:512],
                                     func=AF.Silu)
                nc.scalar.activation(out=sg[:, 512:CAP], in_=pwB[:, 0:64],
                                     func=AF.Silu)
                nc.vector.tensor_mul(guT[:, t, 0:512], sg[:, 0:512],
                                     pwA[:, 512:1024])
                nc.vector.tensor_mul(guT[:, t, 512:CAP], sg[:, 512:CAP],
                                     pwB[:, 64:128])

            for tt in range(NTT):
                ntok = min(128, CAP - tt * 128)
                tsl = slice(tt * 128, tt * 128 + ntok)
                ot = outp.tile([128, D], BF16, tag="ot")
                for half in range(2):
                    dsl = slice(half * 512, half * 512 + 512)
                    pso = po_p.tile([128, 512], F32, tag="o",
                                    name=f"pso{tt}_{half}")
                    for hp in range(H // 256):
                        nc.tensor.matmul(
                            pso[0:ntok, :], guT[:, 2 * hp:2 * hp + 2, tsl],
                            dwn8[:, hp, :, dsl],
                            start=(hp == 0), stop=(hp == H // 256 - 1),
                            perf_mode=PM.DoubleRow)
                    nc.vector.tensor_scalar_mul(ot[0:ntok, dsl],
                                                pso[0:ntok, :],
                                                wts[0:ntok, tt:tt + 1])
                    nc.sync.dma_start(out=eout_d[tsl, dsl],
                                      in_=ot[0:ntok, dsl])
    nc.compile()
    return nc


# --------------------------------------------------------------------------
# Host orchestration
# --------------------------------------------------------------------------

def _get(name, builder):
    if name not in _CACHE:
        _CACHE[name] = builder()
    return _CACHE[name]


def _attn_inputs(x2d, wq, bq, wkv, bkv, wo, norm1_w):
    """Build the 8 per-core input maps for launch A."""
    h = x2d.astype(np.float64)
    h = h / np.sqrt((h * h).mean(axis=-1, keepdims=True) + EPS)
    h = (h * norm1_w.astype(np.float64)).astype(np.float32)
    # hT[p, c, t] = h[t, 128c+p]
    hT = np.ascontiguousarray(
        h.T.reshape(D // 128, 128, T).transpose(1, 0, 2).astype(BF16_NP))

    wk = wkv[:, :D]
    wv = wkv[:, D:]
    bk = bkv[:D]

    tk = np.arange(128)[:, None]
    u = np.arange(128)[None, :]
    trimask = (u >= tk).astype(BF16_NP)
    onesb = np.ones((1, 128), BF16_NP)

    ins = []
    for c in range(NCORES):
        cs = slice(c * CW, (c + 1) * CW)
        packed = {n: np.ascontiguousarray(
            w[:, cs].reshape(D // 128, 128, CW).transpose(1, 0, 2)
            .astype(BF16_NP)) for n, w in (("wq", wq), ("wk", wk),
                                           ("wv", wv))}
        bqk_c = np.ascontiguousarray(
            np.stack([bq[cs], bk[cs]], axis=1).astype(np.float32))
        wo_c = np.ascontiguousarray(wo[cs, :].astype(BF16_NP))
        ins.append({
            "hT": hT,
            **packed,
            "bqk": bqk_c,
            "wo": wo_c,
            "trimask": trimask,
            "onesb": onesb,
        })
    return ins


def _route(x2, router_w, norm2_w):
    """Exact reference routing on host: rmsnorm2 + top-2 + softmax."""
    h2 = x2 / np.sqrt(np.mean(x2 * x2, axis=-1, keepdims=True) + EPS)
    h2 = (h2 * norm2_w).astype(np.float32)
    logits = h2.astype(np.float32) @ router_w.astype(np.float32)   # [N, E]
    idx1 = np.argmax(logits, axis=-1)
    l2 = logits.copy()
    l2[np.arange(T), idx1] = -np.inf
    idx2 = np.argmax(l2, axis=-1)
    v1 = logits[np.arange(T), idx1]
    v2 = logits[np.arange(T), idx2]
    # softmax over the two selected logits (v1 >= v2)
    e2 = np.exp((v2 - v1).astype(np.float32))
    p1 = (1.0 / (1.0 + e2)).astype(np.float32)
    p2 = (e2 / (1.0 + e2)).astype(np.float32)
    return h2, idx1, idx2, p1, p2


def kernel(x, freqs_cos, freqs_sin, norm1_w, wq, bq, wkv, bkv, wo, bo,
           norm2_w, router_w, gate_w, up_w, down_w):
    global MOE_ROUNDS
    x = np.asarray(x, np.float32)
    x2d = np.ascontiguousarray(x.reshape(T, D))
    wq = np.asarray(wq, np.float32)
    wkv = np.asarray(wkv, np.float32)
    wo = np.asarray(wo, np.float32)
    bq = np.asarray(bq, np.float32)
    bkv = np.asarray(bkv, np.float32)
    bo = np.asarray(bo, np.float32)
    norm1_w = np.asarray(norm1_w, np.float32)
    norm2_w = np.asarray(norm2_w, np.float32)
    router_w = np.asarray(router_w, np.float32)
    gate_w = np.asarray(gate_w, np.float32)
    up_w = np.asarray(up_w, np.float32)
    down_w = np.asarray(down_w, np.float32)

    # ---- launch A ----
    nc_a = _get("attn", build_attn)
    ins_a = _attn_inputs(x2d, wq, bq, wkv, bkv, wo, norm1_w)
    res_a = run_bass_kernel_spmd(nc_a, ins_a, core_ids=list(range(NCORES)))
    parts = np.stack([res_a.results[c]["part"].astype(np.float64)
                      for c in range(NCORES)])
    # v-bias folds through attention as +bv (softmax weights sum to 1),
    # so its wo image is added host-side along with bo.
    bv = bkv[D:].astype(np.float64)
    x2 = (x2d.astype(np.float64) + parts.sum(axis=0)
          + bv @ wo.astype(np.float64) + bo.astype(np.float64)
          ).astype(np.float32)

    # ---- host routing ----
    h2, idx1, idx2, p1, p2 = _route(x2, router_w, norm2_w)

    # per-expert token lists (order: top-1 hits then top-2 hits, stable)
    work = []   # (expert, token_idx array, weight array)
    for e in range(E):
        m1 = idx1 == e
        m2 = idx2 == e
        toks = np.concatenate([np.nonzero(m1)[0], np.nonzero(m2)[0]])
        wgts = np.concatenate([p1[m1], p2[m2]]).astype(np.float32)
        for s in range(0, max(len(toks), 1), CAP):
            work.append((e, toks[s:s + CAP], wgts[s:s + CAP]))

    h28 = h2.astype(F8_NP)
    guwb: dict = {}
    dwnb: dict = {}
    NTT = (CAP + 127) // 128

    # ---- launch B (one round of 8 unless an expert overflows CAP) ----
    nc_b = _get("moe", build_moe)
    moe = np.zeros((T, D), np.float64)
    MOE_ROUNDS = 0
    for r0 in range(0, len(work), NCORES):
        batch = work[r0:r0 + NCORES]
        while len(batch) < NCORES:
            batch.append((0, np.zeros(0, np.int64), np.zeros(0, np.float32)))
        ins_b = []
        for e, toks, wgts in batch:
            tok8 = np.zeros((128, D // 128, CAP), F8_NP)
            tok8t = h28[toks].T.reshape(D // 128, 128, len(toks))
            tok8[:, :, :len(toks)] = tok8t.transpose(1, 0, 2)
            wts = np.zeros((NTT * 128,), np.float32)
            wts[:len(toks)] = wgts * MOE_SCALE
            if e not in guwb:
                gu = np.concatenate([
                    gate_w[e].reshape(D, H // 128, 128),
                    up_w[e].reshape(D, H // 128, 128)], axis=2)  # [D,16,256]
                guwb[e] = np.ascontiguousarray(
                    gu.reshape(D // 128, 128, H // 128, 256)
                    .transpose(2, 1, 0, 3).astype(F8_NP))
                dwnb[e] = np.ascontiguousarray(
                    down_w[e].reshape(H // 256, 2, 128, D)
                    .transpose(2, 0, 1, 3).astype(F8_NP))
            ins_b.append({
                "tok8": tok8,
                "guw": guwb[e],
                "dwn8": dwnb[e],
                "wts": np.ascontiguousarray(
                    wts.reshape(NTT, 128).T.astype(np.float32)),
            })
        res_b = run_bass_kernel_spmd(nc_b, ins_b, core_ids=list(range(NCORES)))
        MOE_ROUNDS += 1
        for (e, toks, wgts), rc in zip(batch, res_b.results):
            if len(toks):
                moe[toks] += rc["eout"][:len(toks)].astype(np.float64)

    out = (x2.astype(np.float64) + moe).astype(np.float32)
    return out.reshape(B, T, D)
